# revision 14
# baseline (speedup 1.0000x reference)
"""nn_AffineLog: batched 4x4 affine matrix-log projected onto the 7-dim CSO basis.

Closed-form algorithm (replaces the reference's 24-term Mercator series):
inputs are exactly [[e^s R, t],[0,1]] with R a rotation, so
  L3x3 = s I + f (R - R^T),  f = asin(x)/(2x), x = sin th  (poly in x^2)
  translation u' = psi(C) t, psi(x) = x/(e^x-1), reduced via
  Omega^3 = -th^2 Omega to u' = (A - D q) t + B (w x t) + D (w.t) w.

Data-parallel over 8 NeuronCores. The host packs the 10 live channels of
each affine into channel-planar per-partition DRAM, so every DVE access is
contiguous; elementwise pipeline on DVE/ACT with custom fused DVE ops.
"""

import os

os.environ.setdefault("BY_DEFAULT_DISABLE_SUBTILE_DEPS", "1")

import functools

import numpy as np

import concourse.bass as bass
import concourse.bacc as bacc
import concourse.hw_specs as hw_specs
import concourse.mybir as mybir
from concourse.tile import TileContext
from concourse.tile_rust import add_dep_helper
from concourse.bass_utils import run_bass_kernel_spmd
from concourse import dve_ops as dops
from concourse.dve_spec import (
    Spec, Src0, Src1, C0, C1, C2, C3, One, sq, _spill_c3_to_src1, lower,
    _has_src1,
)
from concourse.dve_uop import DveOpSpec

AF = mybir.ActivationFunctionType
OP = mybir.AluOpType
F32 = mybir.dt.float32

NCORES = 8
B = 2_000_000
P = 128
JPP = 1954                  # free-dim elements per partition per core
NC_ELEMS = P * JPP          # 250112 per core (total 2000896, pad 896)
TILES = (226, 768, 768, 192)

# packed channel order (host): [m01, m10, m02, m20, m12, m21, m00] + [t0, t1, t2]
CH_A = [1, 4, 2, 8, 6, 9, 0]   # 7 "matrix" planes -> tensor xa
CH_B = [3, 7, 11]              # 3 translation planes -> tensor xb

SQ2 = float(np.sqrt(2.0))
SQ3 = float(np.sqrt(3.0))
# f'(z) = 2*asin(x)/(2x) with z = 4x^2:  f' = 1 + c1 z + c2 z^2 + c3 z^3 + c4 z^4
FP_C1 = 1.0 / 24.0
FP_C2 = 2.0 * 0.5 * (3.0 / 40.0) / 16.0
FP_C3 = 2.0 * 0.5 * (5.0 / 112.0) / 64.0
FP_C4 = 2.0 * 0.5 * (35.0 / 1152.0) / 256.0

# Restrict ACT table choice to the one set holding ln+exp+copy, so bacc
# never alternates table loads between tiles. Other set names stay (ids are
# positional) but advertise no functions.
_orig_gat = hw_specs.get_activation_tables


@functools.cache
def _gat_ln_exp_only(module_arch):
    t = _orig_gat(module_arch)
    keep = "natural_log_exp_and_others"
    return {k: (v if k == keep else set()) for k, v in t.items()}


hw_specs.get_activation_tables = _gat_ln_exp_only
bacc.get_activation_tables = _gat_ln_exp_only


# --- custom fused DVE ops (registered into concourse.dve_ops at import) ----
def _register(name, body):
    if name in dops._SUB_OPCODE_FOR_NAME:
        return next(o for o in dops.OPS if o.name == name)
    dops._SUB_OPCODE_FOR_NAME[name] = dops._CUSTOM_DVE_ROW_BASE + len(dops.OPS)
    assert dops._SUB_OPCODE_FOR_NAME[name] < 0x20
    spec = Spec(body=body)
    lowered = DveOpSpec(
        name=name,
        opcode=dops._SUB_OPCODE_FOR_NAME[name],
        uops=lower(spec, ver="v3"),
        rd1_en=_has_src1(spec),
    )
    op = dops.DveOp(name=name, spec=spec, subdim=False,
                    uops_sha={"v3": lowered.sha("v3")})
    dops.OPS.append(op)
    dops.CUSTOM_DVE_SPECS[name] = spec
    return op


OP_SQSUM = _register("ANT_AFL_SQSUM", sq(Src0) + sq(Src1))
OP_ADDSQ = _register("ANT_AFL_ADDSQ", Src0 + sq(Src1))
OP_POLY4 = _register(
    "ANT_AFL_POLY4",
    _spill_c3_to_src1(((((Src0 * C0 + C1) * Src0 + C2) * Src0 + C3) * Src0) + One),
)
_m2 = (Src0 * C0) * Src0
OP_ACOEF = _register("ANT_AFL_ACOEF", (((_m2 + C1) * Src0 + C2) * Src0) + One)
_s2 = Src0 * Src0
OP_BCOEF = _register(
    "ANT_AFL_BCOEF", (Src0 * C1 + C2) + ((_s2 * Src0 - Src0 * Src1) * C0))
OP_DCOEF = _register(
    "ANT_AFL_DCOEF", ((Src0 * Src0) * C0 + Src1 * C1) + C2)
OP_QTH = _register("ANT_AFL_QTH", (sq(Src0) * Src1) * C0)
OP_DG2 = _register("ANT_AFL_DG2", Src0 * sq(Src1))


def _build(jpp=JPP, tiles=TILES):
    nc = bacc.Bacc("TRN2", target_bir_lowering=False, debug=False)
    xa = nc.dram_tensor("xa", (P, 7 * jpp), F32, kind="ExternalInput")
    xb = nc.dram_tensor("xb", (P, 3 * jpp), F32, kind="ExternalInput")
    ya = nc.dram_tensor("ya", (P, 3 * jpp), F32, kind="ExternalOutput")
    yb = nc.dram_tensor("yb", (P, 4 * jpp), F32, kind="ExternalOutput")
    xav = xa[:, :].rearrange("p (c j) -> p c j", j=jpp)
    xbv = xb[:, :].rearrange("p (c j) -> p c j", j=jpp)
    yav = ya[:, :].rearrange("p (c j) -> p c j", j=jpp)
    ybv = yb[:, :].rearrange("p (c j) -> p c j", j=jpp)

    mul, add, sub = OP.mult, OP.add, OP.subtract

    with TileContext(nc) as tc:
        with (
            tc.tile_pool(name="cst", bufs=1) as cstp,
            tc.tile_pool(name="io", bufs=2) as iop,
            tc.tile_pool(name="tp", bufs=1) as tp,
        ):
            c1col = cstp.tile([P, 1], F32, name="c1col")
            nc.vector.memset(c1col, FP_C1)

            off = 0
            for nf in tiles:
                INA = iop.tile([P, nf * 7], F32, tag="ina", name="tina")
                INB = iop.tile([P, nf * 3], F32, tag="inb", name="tinb")
                OUTB = iop.tile([P, nf * 4], F32, tag="outb", name="toutb")
                nc.sync.dma_start(
                    out=INA.rearrange("p (c j) -> p c j", c=7),
                    in_=xav[:, :, off:off + nf])
                nc.sync.dma_start(
                    out=INB.rearrange("p (c j) -> p c j", c=3),
                    in_=xbv[:, :, off:off + nf])

                def T(nm, k=1):
                    return tp.tile([P, nf * k], F32, tag=nm, name=nm)

                def pl(t, i, k=1):
                    return t[:, i * nf:(i + k) * nf]

                def pl3(t, i=0):
                    return t[:, i * nf:(i + 3) * nf].rearrange(
                        "p (c j) -> p c j", c=3)

                def bc3(a):
                    return a.rearrange("p (o j) -> p o j", o=1).to_broadcast(
                        [P, 3, nf])

                def tt(o, a, b, op):
                    nc.vector.tensor_tensor(out=o, in0=a, in1=b, op=op)

                def stt(o, a, s, b, op0, op1):
                    nc.vector.scalar_tensor_tensor(
                        out=o, in0=a, scalar=s, in1=b, op0=op0, op1=op1)

                def cust(op_, o, a, b=None, s0=0.0, s1=0.0, imm2=0.0):
                    nc.vector._custom_dve(
                        op_, out=o, in0=a, in1=b, s0=s0, s1=s1, imm2=imm2)

                tv = pl3(INB)  # [p, 3, nf] translation planes

                u = T("u"); v = T("v")
                # e^{2s} = m00^2 + m10^2 + m20^2  (planes 6, 1, 3 of INA)
                cust(OP_SQSUM, u, pl(INA, 6), pl(INA, 1))
                e2s = T("e2s")
                cust(OP_ADDSQ, e2s, u, pl(INA, 3))
                lnd2 = T("lnd2"); es = T("es"); es2 = T("es2"); s = T("s")
                nc.scalar.activation(out=lnd2, in_=e2s, func=AF.Ln)
                nc.scalar.activation(out=es, in_=lnd2, func=AF.Exp, scale=-0.5)
                nc.scalar.activation(out=es2, in_=lnd2, func=AF.Exp, scale=-1.0)
                nc.scalar.mul(s, lnd2, 0.5)
                nc.scalar.mul(pl(OUTB, 3), lnd2, SQ3 / 2.0)   # out6

                A3 = tp.tile([P, nf * 3], F32, tag="A3", name="A3", bufs=2)
                tt(pl(A3, 0), pl(INA, 0), pl(INA, 1), sub)   # a1 = m01 - m10
                tt(pl(A3, 1), pl(INA, 2), pl(INA, 3), sub)   # a2 = m02 - m20
                tt(pl(A3, 2), pl(INA, 4), pl(INA, 5), sub)   # a3 = m12 - m21
                cust(OP_SQSUM, v, pl(A3, 0), pl(A3, 1))
                S = T("S")
                cust(OP_ADDSQ, S, v, pl(A3, 2))
                # all 9 products P[i,j] = a_i * t_j at plane 3i+j
                P9 = T("P9", 9)
                for i in range(3):
                    tt(pl3(P9, 3 * i), bc3(pl(A3, i)), tv, mul)
                # dtil first (consumes planes 4,6,2), then ctil into 2,4,6
                dA = T("dA"); dt = T("dt")
                tt(dA, pl(P9, 4), pl(P9, 6), sub)               # a2t1-a3t0
                tt(dt, dA, pl(P9, 2), sub)                      # - a1t2
                tt(pl(P9, 2), pl(P9, 1), pl(P9, 5), add)        # cx
                tt(pl(P9, 4), pl(P9, 8), pl(P9, 0), sub)        # cy
                stt(pl(P9, 6), pl(P9, 7), -1.0, pl(P9, 3), mul, sub)  # cz
                # scalar chain (ACT outputs ready by now)
                z = T("z")
                tt(z, es2, S, mul)                  # z = 4 sin^2 th
                fp = T("fp")
                cust(OP_POLY4, fp, z, c1col, s0=FP_C4, s1=FP_C3, imm2=FP_C2)
                qt = T("qt")
                cust(OP_QTH, qt, fp, z, s0=0.25)    # th^2
                g = T("g")
                stt(g, fp, 0.5, es, mul, mul)       # g = f e^{-s}
                # rotation outputs = sqrt2 * g * a_k -> OUTB planes 0..2
                stt(pl3(OUTB), bc3(g), SQ2, pl3(A3), mul, mul)
                nc.sync.dma_start(
                    out=ybv[:, :, off:off + nf],
                    in_=OUTB.rearrange("p (c j) -> p c j", c=4))
                # psi coefficients (slots reuse dead temps)
                A = T("e2s"); Bc = T("S"); D = T("lnd2")
                cust(OP_ACOEF, A, s,
                     s0=-1.0 / 720.0, s1=1.0 / 12.0, imm2=-0.5)
                cust(OP_BCOEF, Bc, s, qt,
                     s0=-1.0 / 180.0, s1=1.0 / 6.0, imm2=-0.5)
                cust(OP_DCOEF, D, s, qt,
                     s0=-1.0 / 120.0, s1=1.0 / 720.0, imm2=1.0 / 12.0)
                v2 = T("u"); Ap = T("es2"); Bg = T("s"); Dg2 = T("fp")
                tt(v2, D, qt, mul)
                tt(Ap, A, v2, sub)
                tt(Bg, Bc, g, mul)
                cust(OP_DG2, Dg2, D, g)
                P3 = T("z")
                tt(P3, Dg2, dt, mul)
                # pw = P3*(-a3,+a2,-a1) into free P9 planes 1,3,5
                stt(pl(P9, 1), P3, -1.0, pl(A3, 2), mul, mul)
                tt(pl(P9, 3), P3, pl(A3, 1), mul)
                stt(pl(P9, 5), P3, -1.0, pl(A3, 0), mul, mul)
                # w1 = Ap*t ; w2 = Bg*ctil (into A3, fully consumed).
                # The 3-term sum w1 + w2 + pw is folded into three
                # accumulating SWDGE DMAs onto the zero-initialised ya —
                # DRAM is not Tile-tracked, so order them explicitly.
                W1 = T("W1", 3)
                tt(pl3(W1), bc3(Ap), tv, mul)
                cview = P9[:, 2 * nf:8 * nf].rearrange(
                    "p (c t j) -> p c t j", c=3, t=2)[:, :, 0, :]
                pwview = P9[:, 1 * nf:7 * nf].rearrange(
                    "p (c t j) -> p c t j", c=3, t=2)[:, :, 0, :]
                tt(pl3(A3), bc3(Bg), cview, mul)
                ydst = yav[:, :, off:off + nf]
                d1 = nc.gpsimd.dma_start(
                    out=ydst, in_=pl3(W1), accum_op=add)
                d2 = nc.gpsimd.dma_start(
                    out=ydst, in_=pl3(A3), accum_op=add)
                d3 = nc.gpsimd.dma_start(
                    out=ydst, in_=pwview, accum_op=add)
                add_dep_helper(d2.ins, d1.ins, sync=True, reason="ya accum order")
                add_dep_helper(d3.ins, d2.ins, sync=True, reason="ya accum order")
                off += nf
    if not nc.is_finalized():
        nc.finalize()
    return nc


def _pack(affine):
    """(B,4,4) f32 -> per-core channel-planar arrays xa (P,7*jpp), xb (P,3*jpp)."""
    x = np.ascontiguousarray(affine.reshape(B, 16).astype(np.float32, copy=False))
    pad = NCORES * NC_ELEMS - B
    padblk = np.zeros((pad, 16), np.float32)
    padblk[:, [0, 5, 10, 15]] = 1.0  # identity affines -> log = 0
    data = np.concatenate([x, padblk], 0).reshape(NCORES, P, JPP, 16)
    da = np.ascontiguousarray(data[:, :, :, CH_A].transpose(0, 1, 3, 2))
    db = np.ascontiguousarray(data[:, :, :, CH_B].transpose(0, 1, 3, 2))
    return (da.reshape(NCORES, P, 7 * JPP), db.reshape(NCORES, P, 3 * JPP))


def _run(affine, trace=False):
    da, db = _pack(np.asarray(affine))
    nc = _build()
    res = run_bass_kernel_spmd(
        nc,
        [{"xa": da[i], "xb": db[i]} for i in range(NCORES)],
        core_ids=list(range(NCORES)),
        trace=trace,
    )
    out = np.empty((NCORES, P, JPP, 7), np.float32)
    for i, r in enumerate(res.results):
        out[i, :, :, 0:3] = r["ya"].reshape(P, 3, JPP).transpose(0, 2, 1)
        out[i, :, :, 3:7] = r["yb"].reshape(P, 4, JPP).transpose(0, 2, 1)
    return out.reshape(NCORES * NC_ELEMS, 7)[:B], res


def kernel(affine):
    y, _ = _run(np.asarray(affine), trace=False)
    return y


# revision 23
# speedup vs baseline: 1.1553x; 1.1553x over previous
"""nn_AffineLog: batched 4x4 affine matrix-log projected onto the 7-dim CSO basis.

Closed-form algorithm (replaces the reference's 24-term Mercator series):
inputs are exactly [[e^s R, t],[0,1]] with R a rotation, so
  L3x3 = s I + f (R - R^T),  f = asin(x)/(2x), x = sin th  (poly in x^2)
  translation u' = psi(C) t, psi(x) = x/(e^x-1), reduced via
  Omega^3 = -th^2 Omega to u' = (A - D q) t + B (w x t) + D (w.t) w.

Data-parallel over 8 NeuronCores. The host packs the 10 live channels of
each affine into channel-planar per-partition DRAM, so every DVE access is
contiguous; elementwise pipeline on DVE/ACT with custom fused DVE ops.
"""

import os

os.environ.setdefault("BY_DEFAULT_DISABLE_SUBTILE_DEPS", "1")

import functools
from contextlib import ExitStack

import numpy as np

import concourse.bass as bass
import concourse.bacc as bacc
import concourse.hw_specs as hw_specs
import concourse.mybir as mybir
from concourse.tile import TileContext
from concourse.bass_utils import run_bass_kernel_spmd
from concourse import dve_ops as dops
from concourse.dve_spec import (
    Spec, Src0, Src1, C0, C1, C2, C3, One, sq, _spill_c3_to_src1, lower,
    _has_src1,
)
from concourse.dve_uop import DveOpSpec

AF = mybir.ActivationFunctionType
OP = mybir.AluOpType
F32 = mybir.dt.float32

NCORES = 8
B = 2_000_000
P = 128
JPP = 1954                  # free-dim elements per partition per core
NC_ELEMS = P * JPP          # 250112 per core (total 2000896, pad 896)
TILES = (226, 768, 768, 192)

# packed channel order (host): [m01, m10, m02, m20, m12, m21, m00] + [t0, t1, t2]
CH_A = [1, 4, 2, 8, 6, 9, 0]   # 7 "matrix" planes -> tensor xa
CH_B = [3, 7, 11]              # 3 translation planes -> tensor xb

SQ2 = float(np.sqrt(2.0))
SQ3 = float(np.sqrt(3.0))
# f'(z) = 2*asin(x)/(2x) with z = 4x^2:  f' = 1 + c1 z + c2 z^2 + c3 z^3 + c4 z^4
FP_C1 = 1.0 / 24.0
FP_C2 = 2.0 * 0.5 * (3.0 / 40.0) / 16.0
FP_C3 = 2.0 * 0.5 * (5.0 / 112.0) / 64.0
FP_C4 = 2.0 * 0.5 * (35.0 / 1152.0) / 256.0

# Restrict ACT table choice to the one set holding ln+exp+copy, so bacc
# never alternates table loads between tiles. Other set names stay (ids are
# positional) but advertise no functions.
_orig_gat = hw_specs.get_activation_tables


@functools.cache
def _gat_ln_exp_only(module_arch):
    t = _orig_gat(module_arch)
    keep = "natural_log_exp_and_others"
    return {k: (v if k == keep else set()) for k, v in t.items()}


hw_specs.get_activation_tables = _gat_ln_exp_only
bacc.get_activation_tables = _gat_ln_exp_only


# --- custom fused DVE ops (registered into concourse.dve_ops at import) ----
def _register(name, body):
    if name in dops._SUB_OPCODE_FOR_NAME:
        return next(o for o in dops.OPS if o.name == name)
    dops._SUB_OPCODE_FOR_NAME[name] = dops._CUSTOM_DVE_ROW_BASE + len(dops.OPS)
    assert dops._SUB_OPCODE_FOR_NAME[name] < 0x20
    spec = Spec(body=body)
    lowered = DveOpSpec(
        name=name,
        opcode=dops._SUB_OPCODE_FOR_NAME[name],
        uops=lower(spec, ver="v3"),
        rd1_en=_has_src1(spec),
    )
    op = dops.DveOp(name=name, spec=spec, subdim=False,
                    uops_sha={"v3": lowered.sha("v3")})
    dops.OPS.append(op)
    dops.CUSTOM_DVE_SPECS[name] = spec
    return op


OP_SQSUM = _register("ANT_AFL_SQSUM", sq(Src0) + sq(Src1))
OP_ADDSQ = _register("ANT_AFL_ADDSQ", Src0 + sq(Src1))
OP_POLY4 = _register(
    "ANT_AFL_POLY4",
    _spill_c3_to_src1(((((Src0 * C0 + C1) * Src0 + C2) * Src0 + C3) * Src0) + One),
)
_m2 = (Src0 * C0) * Src0
OP_ACOEF = _register("ANT_AFL_ACOEF", (((_m2 + C1) * Src0 + C2) * Src0) + One)
_s2 = Src0 * Src0
OP_BCOEF = _register(
    "ANT_AFL_BCOEF", (Src0 * C1 + C2) + ((_s2 * Src0 - Src0 * Src1) * C0))
OP_DCOEF = _register(
    "ANT_AFL_DCOEF", ((Src0 * Src0) * C0 + Src1 * C1) + C2)
OP_QTH = _register("ANT_AFL_QTH", (sq(Src0) * Src1) * C0)
OP_DG2 = _register("ANT_AFL_DG2", Src0 * sq(Src1))


def _build(jpp=JPP, tiles=TILES):
    nc = bacc.Bacc("TRN2", target_bir_lowering=False, debug=False)
    xa = nc.dram_tensor("xa", (P, 7 * jpp), F32, kind="ExternalInput")
    xb = nc.dram_tensor("xb", (P, 3 * jpp), F32, kind="ExternalInput")
    ident = nc.dram_tensor("ident", (P, P), F32, kind="ExternalInput")
    ya = nc.dram_tensor("ya", (P, 3 * jpp), F32, kind="ExternalOutput")
    yb = nc.dram_tensor("yb", (P, 4 * jpp), F32, kind="ExternalOutput")
    xav = xa[:, :].rearrange("p (c j) -> p c j", j=jpp)
    xbv = xb[:, :].rearrange("p (c j) -> p c j", j=jpp)
    yav = ya[:, :].rearrange("p (c j) -> p c j", j=jpp)
    ybv = yb[:, :].rearrange("p (c j) -> p c j", j=jpp)

    mul, add, sub = OP.mult, OP.add, OP.subtract

    with TileContext(nc) as tc:
        with (
            tc.tile_pool(name="cst", bufs=1) as cstp,
            tc.tile_pool(name="io", bufs=2) as iop,
            tc.tile_pool(name="tp", bufs=1) as tp,
            tc.tile_pool(name="ps", bufs=1, space="PSUM") as psp,
        ):
            c1col = cstp.tile([P, 1], F32, name="c1col")
            nc.vector.memset(c1col, FP_C1)
            IDT = cstp.tile([P, P], F32, name="IDT")
            nc.sync.dma_start(out=IDT, in_=ident[:, :])

            off = 0
            for nf in tiles:
                INA = iop.tile([P, nf * 7], F32, tag="ina", name="tina")
                INB = iop.tile([P, nf * 3], F32, tag="inb", name="tinb")
                OUTA = iop.tile([P, nf * 3], F32, tag="outa", name="touta")
                OUTB = iop.tile([P, nf * 4], F32, tag="outb", name="toutb")
                nc.sync.dma_start(
                    out=INA.rearrange("p (c j) -> p c j", c=7),
                    in_=xav[:, :, off:off + nf])
                nc.sync.dma_start(
                    out=INB.rearrange("p (c j) -> p c j", c=3),
                    in_=xbv[:, :, off:off + nf])

                def T(nm, k=1):
                    return tp.tile([P, nf * k], F32, tag=nm, name=nm)

                def pl(t, i, k=1):
                    return t[:, i * nf:(i + k) * nf]

                def pl3(t, i=0):
                    return t[:, i * nf:(i + 3) * nf].rearrange(
                        "p (c j) -> p c j", c=3)

                def bc3(a):
                    return a.rearrange("p (o j) -> p o j", o=1).to_broadcast(
                        [P, 3, nf])

                def tt(o, a, b, op):
                    nc.vector.tensor_tensor(out=o, in0=a, in1=b, op=op)

                def stt(o, a, s, b, op0, op1):
                    nc.vector.scalar_tensor_tensor(
                        out=o, in0=a, scalar=s, in1=b, op0=op0, op1=op1)

                def cust(op_, o, a, b=None, s0=0.0, s1=0.0, imm2=0.0):
                    nc.vector._custom_dve(
                        op_, out=o, in0=a, in1=b, s0=s0, s1=s1, imm2=imm2)

                tv = pl3(INB)  # [p, 3, nf] translation planes

                u = T("u"); v = T("v")
                # e^{2s} = m00^2 + m10^2 + m20^2  (planes 6, 1, 3 of INA)
                cust(OP_SQSUM, u, pl(INA, 6), pl(INA, 1))
                e2s = T("e2s")
                cust(OP_ADDSQ, e2s, u, pl(INA, 3))
                lnd2 = T("lnd2"); es = T("es"); es2 = T("es2"); s = T("s")
                nc.scalar.activation(out=lnd2, in_=e2s, func=AF.Ln)
                nc.scalar.activation(out=es, in_=lnd2, func=AF.Exp, scale=-0.5)
                nc.scalar.activation(out=es2, in_=lnd2, func=AF.Exp, scale=-1.0)
                nc.scalar.mul(s, lnd2, 0.5)
                nc.scalar.mul(pl(OUTB, 3), lnd2, SQ3 / 2.0)   # out6

                A3 = T("A3", 3)
                tt(pl(A3, 0), pl(INA, 0), pl(INA, 1), sub)   # a1 = m01 - m10
                tt(pl(A3, 1), pl(INA, 2), pl(INA, 3), sub)   # a2 = m02 - m20
                tt(pl(A3, 2), pl(INA, 4), pl(INA, 5), sub)   # a3 = m12 - m21
                cust(OP_SQSUM, v, pl(A3, 0), pl(A3, 1))
                S = T("S")
                cust(OP_ADDSQ, S, v, pl(A3, 2))
                # all 9 products P[i,j] = a_i * t_j at plane 3i+j
                P9 = T("P9", 9)
                for i in range(3):
                    tt(pl3(P9, 3 * i), bc3(pl(A3, i)), tv, mul)
                # dtil first (consumes planes 4,6,2), then ctil into 2,4,6
                dA = T("dA"); dt = T("dt")
                tt(dA, pl(P9, 4), pl(P9, 6), sub)               # a2t1-a3t0
                tt(dt, dA, pl(P9, 2), sub)                      # - a1t2
                tt(pl(P9, 2), pl(P9, 1), pl(P9, 5), add)        # cx
                tt(pl(P9, 4), pl(P9, 8), pl(P9, 0), sub)        # cy
                stt(pl(P9, 6), pl(P9, 7), -1.0, pl(P9, 3), mul, sub)  # cz
                # scalar chain (ACT outputs ready by now)
                z = T("z")
                tt(z, es2, S, mul)                  # z = 4 sin^2 th
                fp = T("fp")
                cust(OP_POLY4, fp, z, c1col, s0=FP_C4, s1=FP_C3, imm2=FP_C2)
                qt = T("qt")
                cust(OP_QTH, qt, fp, z, s0=0.25)    # th^2
                g = T("g")
                stt(g, fp, 0.5, es, mul, mul)       # g = f e^{-s}
                # rotation outputs = sqrt2 * g * a_k -> OUTB planes 0..2
                stt(pl3(OUTB), bc3(g), SQ2, pl3(A3), mul, mul)
                nc.sync.dma_start(
                    out=ybv[:, :, off:off + nf],
                    in_=OUTB.rearrange("p (c j) -> p c j", c=4))
                # psi coefficients (slots reuse dead temps)
                A = T("e2s"); Bc = T("S"); D = T("lnd2")
                cust(OP_ACOEF, A, s,
                     s0=-1.0 / 720.0, s1=1.0 / 12.0, imm2=-0.5)
                cust(OP_BCOEF, Bc, s, qt,
                     s0=-1.0 / 180.0, s1=1.0 / 6.0, imm2=-0.5)
                cust(OP_DCOEF, D, s, qt,
                     s0=-1.0 / 120.0, s1=1.0 / 720.0, imm2=1.0 / 12.0)
                v2 = T("u"); Ap = T("es2"); Bg = T("s"); Dg2 = T("fp")
                tt(v2, D, qt, mul)
                tt(Ap, A, v2, sub)
                tt(Bg, Bc, g, mul)
                cust(OP_DG2, Dg2, D, g)
                P3 = T("z")
                tt(P3, Dg2, dt, mul)
                # pw = P3*(-a3,+a2,-a1) into free P9 planes 1,3,5
                stt(pl(P9, 1), P3, -1.0, pl(A3, 2), mul, mul)
                tt(pl(P9, 3), P3, pl(A3, 1), mul)
                stt(pl(P9, 5), P3, -1.0, pl(A3, 0), mul, mul)
                # w1 = Ap*t ; w2 = Bg*ctil (into A3, fully consumed)
                W1 = T("W1", 3)
                tt(pl3(W1), bc3(Ap), tv, mul)
                cview = P9[:, 2 * nf:8 * nf].rearrange(
                    "p (c t j) -> p c t j", c=3, t=2)[:, :, 0, :]
                pwview = P9[:, 1 * nf:7 * nf].rearrange(
                    "p (c t j) -> p c t j", c=3, t=2)[:, :, 0, :]
                tt(pl3(A3), bc3(Bg), cview, mul)
                # sum w1 + w2 + pw on the (idle) TensorEngine: identity-weight
                # matmuls accumulate the three terms per plane in PSUM, ACT
                # copies PSUM -> SBUF. Frees 6 DVE units per tile.
                for i in range(3):
                    PSi = psp.tile([P, nf], F32, tag=f"ps{i}", name=f"ps{i}")
                    pwsrc = pl(P9, 1 + 2 * i)
                    terms = (pl(W1, i), pl(A3, i), pwsrc)
                    for c0 in range(0, nf, 512):
                        w = min(512, nf - c0)
                        for k, src in enumerate(terms):
                            nc.tensor.matmul(
                                PSi[:, c0:c0 + w],
                                IDT[:, :], src[:, c0:c0 + w],
                                start=(k == 0), stop=(k == 2))
                    nc.scalar.copy(pl(OUTA, i), PSi[:, :])
                nc.sync.dma_start(
                    out=yav[:, :, off:off + nf],
                    in_=OUTA.rearrange("p (c j) -> p c j", c=3))
                off += nf
    if not nc.is_finalized():
        nc.finalize()
    return nc


def _pack(affine):
    """(B,4,4) f32 -> per-core channel-planar arrays xa (P,7*jpp), xb (P,3*jpp)."""
    x = np.ascontiguousarray(affine.reshape(B, 16).astype(np.float32, copy=False))
    pad = NCORES * NC_ELEMS - B
    padblk = np.zeros((pad, 16), np.float32)
    padblk[:, [0, 5, 10, 15]] = 1.0  # identity affines -> log = 0
    data = np.concatenate([x, padblk], 0).reshape(NCORES, P, JPP, 16)
    da = np.ascontiguousarray(data[:, :, :, CH_A].transpose(0, 1, 3, 2))
    db = np.ascontiguousarray(data[:, :, :, CH_B].transpose(0, 1, 3, 2))
    return (da.reshape(NCORES, P, 7 * JPP), db.reshape(NCORES, P, 3 * JPP))


def _run(affine, trace=False):
    da, db = _pack(np.asarray(affine))
    nc = _build()
    eye = np.ascontiguousarray(np.eye(P, dtype=np.float32))
    res = run_bass_kernel_spmd(
        nc,
        [{"xa": da[i], "xb": db[i], "ident": eye} for i in range(NCORES)],
        core_ids=list(range(NCORES)),
        trace=trace,
    )
    out = np.empty((NCORES, P, JPP, 7), np.float32)
    for i, r in enumerate(res.results):
        out[i, :, :, 0:3] = r["ya"].reshape(P, 3, JPP).transpose(0, 2, 1)
        out[i, :, :, 3:7] = r["yb"].reshape(P, 4, JPP).transpose(0, 2, 1)
    return out.reshape(NCORES * NC_ELEMS, 7)[:B], res


def kernel(affine):
    y, _ = _run(np.asarray(affine), trace=False)
    return y


# revision 26
# speedup vs baseline: 1.1648x; 1.0082x over previous
"""nn_AffineLog: batched 4x4 affine matrix-log projected onto the 7-dim CSO basis.

Closed-form algorithm (replaces the reference's 24-term Mercator series):
inputs are exactly [[e^s R, t],[0,1]] with R a rotation, so
  L3x3 = s I + f (R - R^T),  f = asin(x)/(2x), x = sin th  (poly in x^2)
  translation u' = psi(C) t, psi(x) = x/(e^x-1), reduced via
  Omega^3 = -th^2 Omega to u' = (A - D q) t + B (w x t) + D (w.t) w.

Data-parallel over 8 NeuronCores. The host packs the 10 live channels of
each affine into channel-planar per-partition DRAM, so every DVE access is
contiguous; elementwise pipeline on DVE/ACT with custom fused DVE ops.
"""

import os

os.environ.setdefault("BY_DEFAULT_DISABLE_SUBTILE_DEPS", "1")

import functools
from contextlib import ExitStack

import numpy as np

import concourse.bass as bass
import concourse.bacc as bacc
import concourse.hw_specs as hw_specs
import concourse.mybir as mybir
from concourse.tile import TileContext
from concourse.bass_utils import run_bass_kernel_spmd
from concourse import dve_ops as dops
from concourse.dve_spec import (
    Spec, Src0, Src1, C0, C1, C2, C3, One, sq, _spill_c3_to_src1, lower,
    _has_src1,
)
from concourse.dve_uop import DveOpSpec

AF = mybir.ActivationFunctionType
OP = mybir.AluOpType
F32 = mybir.dt.float32

NCORES = 8
B = 2_000_000
P = 128
JPP = 1954                  # free-dim elements per partition per core
NC_ELEMS = P * JPP          # 250112 per core (total 2000896, pad 896)
TILES = (226, 640, 640, 448)

# packed channel order (host): [m01, m10, m02, m20, m12, m21, m00] + [t0, t1, t2]
CH_A = [1, 4, 2, 8, 6, 9, 0]   # 7 "matrix" planes -> tensor xa
CH_B = [3, 7, 11]              # 3 translation planes -> tensor xb

SQ2 = float(np.sqrt(2.0))
SQ3 = float(np.sqrt(3.0))
# f'(z) = 2*asin(x)/(2x) with z = 4x^2:  f' = 1 + c1 z + c2 z^2 + c3 z^3 + c4 z^4
FP_C1 = 1.0 / 24.0
FP_C2 = 2.0 * 0.5 * (3.0 / 40.0) / 16.0
FP_C3 = 2.0 * 0.5 * (5.0 / 112.0) / 64.0
FP_C4 = 2.0 * 0.5 * (35.0 / 1152.0) / 256.0

# Restrict ACT table choice to the one set holding ln+exp+copy, so bacc
# never alternates table loads between tiles. Other set names stay (ids are
# positional) but advertise no functions.
_orig_gat = hw_specs.get_activation_tables


@functools.cache
def _gat_ln_exp_only(module_arch):
    t = _orig_gat(module_arch)
    keep = "natural_log_exp_and_others"
    return {k: (v if k == keep else set()) for k, v in t.items()}


hw_specs.get_activation_tables = _gat_ln_exp_only
bacc.get_activation_tables = _gat_ln_exp_only


# --- custom fused DVE ops (registered into concourse.dve_ops at import) ----
def _register(name, body):
    if name in dops._SUB_OPCODE_FOR_NAME:
        return next(o for o in dops.OPS if o.name == name)
    dops._SUB_OPCODE_FOR_NAME[name] = dops._CUSTOM_DVE_ROW_BASE + len(dops.OPS)
    assert dops._SUB_OPCODE_FOR_NAME[name] < 0x20
    spec = Spec(body=body)
    lowered = DveOpSpec(
        name=name,
        opcode=dops._SUB_OPCODE_FOR_NAME[name],
        uops=lower(spec, ver="v3"),
        rd1_en=_has_src1(spec),
    )
    op = dops.DveOp(name=name, spec=spec, subdim=False,
                    uops_sha={"v3": lowered.sha("v3")})
    dops.OPS.append(op)
    dops.CUSTOM_DVE_SPECS[name] = spec
    return op


OP_SQSUM = _register("ANT_AFL_SQSUM", sq(Src0) + sq(Src1))
OP_ADDSQ = _register("ANT_AFL_ADDSQ", Src0 + sq(Src1))
OP_POLY4 = _register(
    "ANT_AFL_POLY4",
    _spill_c3_to_src1(((((Src0 * C0 + C1) * Src0 + C2) * Src0 + C3) * Src0) + One),
)
_m2 = (Src0 * C0) * Src0
OP_ACOEF = _register("ANT_AFL_ACOEF", (((_m2 + C1) * Src0 + C2) * Src0) + One)
_s2 = Src0 * Src0
OP_BCOEF = _register(
    "ANT_AFL_BCOEF", (Src0 * C1 + C2) + ((_s2 * Src0 - Src0 * Src1) * C0))
OP_DCOEF = _register(
    "ANT_AFL_DCOEF", ((Src0 * Src0) * C0 + Src1 * C1) + C2)
OP_QTH = _register("ANT_AFL_QTH", (sq(Src0) * Src1) * C0)
OP_DG2 = _register("ANT_AFL_DG2", Src0 * sq(Src1))


def _build(jpp=JPP, tiles=TILES):
    nc = bacc.Bacc("TRN2", target_bir_lowering=False, debug=False)
    xa = nc.dram_tensor("xa", (P, 7 * jpp), F32, kind="ExternalInput")
    xb = nc.dram_tensor("xb", (P, 3 * jpp), F32, kind="ExternalInput")
    ident = nc.dram_tensor("ident", (P, P), F32, kind="ExternalInput")
    ya = nc.dram_tensor("ya", (P, 3 * jpp), F32, kind="ExternalOutput")
    yb = nc.dram_tensor("yb", (P, 4 * jpp), F32, kind="ExternalOutput")
    xav = xa[:, :].rearrange("p (c j) -> p c j", j=jpp)
    xbv = xb[:, :].rearrange("p (c j) -> p c j", j=jpp)
    yav = ya[:, :].rearrange("p (c j) -> p c j", j=jpp)
    ybv = yb[:, :].rearrange("p (c j) -> p c j", j=jpp)

    mul, add, sub = OP.mult, OP.add, OP.subtract

    with TileContext(nc) as tc:
        with (
            tc.tile_pool(name="cst", bufs=1) as cstp,
            tc.tile_pool(name="io", bufs=2) as iop,
            tc.tile_pool(name="tp", bufs=1) as tp,
            tc.tile_pool(name="ps", bufs=1, space="PSUM") as psp,
        ):
            c1col = cstp.tile([P, 1], F32, name="c1col")
            nc.vector.memset(c1col, FP_C1)
            IDT = cstp.tile([P, P], F32, name="IDT")
            nc.sync.dma_start(out=IDT, in_=ident[:, :])

            off = 0
            for nf in tiles:
                INA = iop.tile([P, nf * 7], F32, tag="ina", name="tina")
                INB = iop.tile([P, nf * 3], F32, tag="inb", name="tinb")
                OUTA = iop.tile([P, nf * 3], F32, tag="outa", name="touta")
                OUTB = iop.tile([P, nf * 4], F32, tag="outb", name="toutb")
                nc.sync.dma_start(
                    out=INA.rearrange("p (c j) -> p c j", c=7),
                    in_=xav[:, :, off:off + nf])
                nc.sync.dma_start(
                    out=INB.rearrange("p (c j) -> p c j", c=3),
                    in_=xbv[:, :, off:off + nf])

                def T(nm, k=1):
                    return tp.tile([P, nf * k], F32, tag=nm, name=nm)

                def pl(t, i, k=1):
                    return t[:, i * nf:(i + k) * nf]

                def pl3(t, i=0):
                    return t[:, i * nf:(i + 3) * nf].rearrange(
                        "p (c j) -> p c j", c=3)

                def bc3(a):
                    return a.rearrange("p (o j) -> p o j", o=1).to_broadcast(
                        [P, 3, nf])

                def tt(o, a, b, op):
                    nc.vector.tensor_tensor(out=o, in0=a, in1=b, op=op)

                def stt(o, a, s, b, op0, op1):
                    nc.vector.scalar_tensor_tensor(
                        out=o, in0=a, scalar=s, in1=b, op0=op0, op1=op1)

                def cust(op_, o, a, b=None, s0=0.0, s1=0.0, imm2=0.0):
                    nc.vector._custom_dve(
                        op_, out=o, in0=a, in1=b, s0=s0, s1=s1, imm2=imm2)

                tv = pl3(INB)  # [p, 3, nf] translation planes

                u = T("u"); v = T("v")
                # e^{2s} = m00^2 + m10^2 + m20^2  (planes 6, 1, 3 of INA)
                cust(OP_SQSUM, u, pl(INA, 6), pl(INA, 1))
                e2s = T("e2s")
                cust(OP_ADDSQ, e2s, u, pl(INA, 3))
                lnd2 = T("lnd2"); es = T("es"); es2 = T("es2"); s = T("s")
                nc.scalar.activation(out=lnd2, in_=e2s, func=AF.Ln)
                nc.scalar.activation(out=es, in_=lnd2, func=AF.Exp, scale=-0.5)
                nc.scalar.activation(out=es2, in_=lnd2, func=AF.Exp, scale=-1.0)
                nc.scalar.mul(s, lnd2, 0.5)
                nc.scalar.mul(pl(OUTB, 3), lnd2, SQ3 / 2.0)   # out6

                # A3/P9 feed the PE accumulation at tile end; double-buffer
                # them so the next tile's DVE work never waits on PE drain.
                A3 = tp.tile([P, nf * 3], F32, tag="A3", name="A3", bufs=2)
                tt(pl(A3, 0), pl(INA, 0), pl(INA, 1), sub)   # a1 = m01 - m10
                tt(pl(A3, 1), pl(INA, 2), pl(INA, 3), sub)   # a2 = m02 - m20
                tt(pl(A3, 2), pl(INA, 4), pl(INA, 5), sub)   # a3 = m12 - m21
                cust(OP_SQSUM, v, pl(A3, 0), pl(A3, 1))
                S = T("S")
                cust(OP_ADDSQ, S, v, pl(A3, 2))
                # all 9 products P[i,j] = a_i * t_j at plane 3i+j
                P9 = tp.tile([P, nf * 9], F32, tag="P9", name="P9", bufs=2)
                for i in range(3):
                    tt(pl3(P9, 3 * i), bc3(pl(A3, i)), tv, mul)
                # dtil first (consumes planes 4,6,2), then ctil into 2,4,6
                dA = T("dA"); dt = T("dt")
                tt(dA, pl(P9, 4), pl(P9, 6), sub)               # a2t1-a3t0
                tt(dt, dA, pl(P9, 2), sub)                      # - a1t2
                tt(pl(P9, 2), pl(P9, 1), pl(P9, 5), add)        # cx
                tt(pl(P9, 4), pl(P9, 8), pl(P9, 0), sub)        # cy
                stt(pl(P9, 6), pl(P9, 7), -1.0, pl(P9, 3), mul, sub)  # cz
                # scalar chain (ACT outputs ready by now)
                z = T("z")
                tt(z, es2, S, mul)                  # z = 4 sin^2 th
                fp = T("fp")
                cust(OP_POLY4, fp, z, c1col, s0=FP_C4, s1=FP_C3, imm2=FP_C2)
                qt = T("qt")
                cust(OP_QTH, qt, fp, z, s0=0.25)    # th^2
                g = T("g")
                stt(g, fp, 0.5, es, mul, mul)       # g = f e^{-s}
                # rotation outputs = sqrt2 * g * a_k -> OUTB planes 0..2
                stt(pl3(OUTB), bc3(g), SQ2, pl3(A3), mul, mul)
                nc.sync.dma_start(
                    out=ybv[:, :, off:off + nf],
                    in_=OUTB.rearrange("p (c j) -> p c j", c=4))
                # psi coefficients (slots reuse dead temps)
                A = T("e2s"); Bc = T("S"); D = T("lnd2")
                cust(OP_ACOEF, A, s,
                     s0=-1.0 / 720.0, s1=1.0 / 12.0, imm2=-0.5)
                cust(OP_BCOEF, Bc, s, qt,
                     s0=-1.0 / 180.0, s1=1.0 / 6.0, imm2=-0.5)
                cust(OP_DCOEF, D, s, qt,
                     s0=-1.0 / 120.0, s1=1.0 / 720.0, imm2=1.0 / 12.0)
                v2 = T("u"); Ap = T("es2"); Bg = T("s"); Dg2 = T("fp")
                tt(v2, D, qt, mul)
                tt(Ap, A, v2, sub)
                tt(Bg, Bc, g, mul)
                cust(OP_DG2, Dg2, D, g)
                P3 = T("z")
                tt(P3, Dg2, dt, mul)
                # pw = P3*(-a3,+a2,-a1) into free P9 planes 1,3,5
                stt(pl(P9, 1), P3, -1.0, pl(A3, 2), mul, mul)
                tt(pl(P9, 3), P3, pl(A3, 1), mul)
                stt(pl(P9, 5), P3, -1.0, pl(A3, 0), mul, mul)
                # w1 = Ap*t ; w2 = Bg*ctil (into A3, fully consumed)
                W1 = T("W1", 3)
                tt(pl3(W1), bc3(Ap), tv, mul)
                cview = P9[:, 2 * nf:8 * nf].rearrange(
                    "p (c t j) -> p c t j", c=3, t=2)[:, :, 0, :]
                pwview = P9[:, 1 * nf:7 * nf].rearrange(
                    "p (c t j) -> p c t j", c=3, t=2)[:, :, 0, :]
                tt(pl3(A3), bc3(Bg), cview, mul)
                # sum w1 + w2 + pw on the (idle) TensorEngine: identity-weight
                # matmuls accumulate the three terms per plane in PSUM, ACT
                # copies PSUM -> SBUF. Frees 6 DVE units per tile.
                for i in range(3):
                    PSi = psp.tile([P, nf], F32, tag=f"ps{i}", name=f"ps{i}")
                    pwsrc = pl(P9, 1 + 2 * i)
                    terms = (pl(W1, i), pl(A3, i), pwsrc)
                    for c0 in range(0, nf, 512):
                        w = min(512, nf - c0)
                        for k, src in enumerate(terms):
                            nc.tensor.matmul(
                                PSi[:, c0:c0 + w],
                                IDT[:, :], src[:, c0:c0 + w],
                                start=(k == 0), stop=(k == 2))
                    nc.scalar.copy(pl(OUTA, i), PSi[:, :])
                nc.sync.dma_start(
                    out=yav[:, :, off:off + nf],
                    in_=OUTA.rearrange("p (c j) -> p c j", c=3))
                off += nf
    if not nc.is_finalized():
        nc.finalize()
    return nc


def _pack(affine):
    """(B,4,4) f32 -> per-core channel-planar arrays xa (P,7*jpp), xb (P,3*jpp)."""
    x = np.ascontiguousarray(affine.reshape(B, 16).astype(np.float32, copy=False))
    pad = NCORES * NC_ELEMS - B
    padblk = np.zeros((pad, 16), np.float32)
    padblk[:, [0, 5, 10, 15]] = 1.0  # identity affines -> log = 0
    data = np.concatenate([x, padblk], 0).reshape(NCORES, P, JPP, 16)
    da = np.ascontiguousarray(data[:, :, :, CH_A].transpose(0, 1, 3, 2))
    db = np.ascontiguousarray(data[:, :, :, CH_B].transpose(0, 1, 3, 2))
    return (da.reshape(NCORES, P, 7 * JPP), db.reshape(NCORES, P, 3 * JPP))


def _run(affine, trace=False):
    da, db = _pack(np.asarray(affine))
    nc = _build()
    eye = np.ascontiguousarray(np.eye(P, dtype=np.float32))
    res = run_bass_kernel_spmd(
        nc,
        [{"xa": da[i], "xb": db[i], "ident": eye} for i in range(NCORES)],
        core_ids=list(range(NCORES)),
        trace=trace,
    )
    out = np.empty((NCORES, P, JPP, 7), np.float32)
    for i, r in enumerate(res.results):
        out[i, :, :, 0:3] = r["ya"].reshape(P, 3, JPP).transpose(0, 2, 1)
        out[i, :, :, 3:7] = r["yb"].reshape(P, 4, JPP).transpose(0, 2, 1)
    return out.reshape(NCORES * NC_ELEMS, 7)[:B], res


def kernel(affine):
    y, _ = _run(np.asarray(affine), trace=False)
    return y


# revision 30
# speedup vs baseline: 1.3549x; 1.1632x over previous
"""nn_AffineLog: batched 4x4 affine matrix-log projected onto the 7-dim CSO basis.

Closed-form algorithm (replaces the reference's 24-term Mercator series):
inputs are exactly [[e^s R, t],[0,1]] with R a rotation, so
  L3x3 = s I + f (R - R^T),  f = asin(x)/(2x), x = sin th  (poly in x^2)
  translation u' = psi(C) t, psi(x) = x/(e^x-1), reduced via
  Omega^3 = -th^2 Omega to u' = (A - D q) t + B (w x t) + D (w.t) w.

Data-parallel over 8 NeuronCores. The host packs the 10 live channels of
each affine into channel-planar per-partition DRAM, so every DVE access is
contiguous; elementwise pipeline on DVE/ACT with custom fused DVE ops.
"""

import os

os.environ.setdefault("BY_DEFAULT_DISABLE_SUBTILE_DEPS", "1")

import functools
from contextlib import ExitStack

import numpy as np

import concourse.bass as bass
import concourse.bacc as bacc
import concourse.hw_specs as hw_specs
import concourse.mybir as mybir
from concourse.tile import TileContext
from concourse.bass_utils import run_bass_kernel_spmd
from concourse import dve_ops as dops
from concourse.dve_spec import (
    Spec, Src0, Src1, C0, C1, C2, C3, One, sq, _spill_c3_to_src1, lower,
    _has_src1,
)
from concourse.dve_uop import DveOpSpec

AF = mybir.ActivationFunctionType
OP = mybir.AluOpType
F32 = mybir.dt.float32

NCORES = 8
B = 2_000_000
P = 128
JPP = 1954                  # free-dim elements per partition per core
NC_ELEMS = P * JPP          # 250112 per core (total 2000896, pad 896)
TILES = (226, 648, 648, 432)

# packed channel order (host): [m01, m10, m02, m20, m12, m21, m00] + [t0, t1, t2]
CH_A = [1, 4, 2, 8, 6, 9, 0]   # 7 "matrix" planes -> tensor xa
CH_B = [3, 7, 11]              # 3 translation planes -> tensor xb

SQ2 = float(np.sqrt(2.0))
SQ3 = float(np.sqrt(3.0))
# f'(z) = 2*asin(x)/(2x) with z = 4x^2:  f' = 1 + c1 z + c2 z^2 + c3 z^3 + c4 z^4
FP_C1 = 1.0 / 24.0
FP_C2 = 2.0 * 0.5 * (3.0 / 40.0) / 16.0
FP_C3 = 2.0 * 0.5 * (5.0 / 112.0) / 64.0
FP_C4 = 2.0 * 0.5 * (35.0 / 1152.0) / 256.0

# Restrict ACT table choice to the one set holding ln+exp+copy, so bacc
# never alternates table loads between tiles. Other set names stay (ids are
# positional) but advertise no functions.
_orig_gat = hw_specs.get_activation_tables


@functools.cache
def _gat_ln_exp_only(module_arch):
    t = _orig_gat(module_arch)
    keep = "natural_log_exp_and_others"
    return {k: (v if k == keep else set()) for k, v in t.items()}


hw_specs.get_activation_tables = _gat_ln_exp_only
bacc.get_activation_tables = _gat_ln_exp_only


# --- custom fused DVE ops (registered into concourse.dve_ops at import) ----
def _register(name, body):
    if name in dops._SUB_OPCODE_FOR_NAME:
        return next(o for o in dops.OPS if o.name == name)
    dops._SUB_OPCODE_FOR_NAME[name] = dops._CUSTOM_DVE_ROW_BASE + len(dops.OPS)
    assert dops._SUB_OPCODE_FOR_NAME[name] < 0x20
    spec = Spec(body=body)
    lowered = DveOpSpec(
        name=name,
        opcode=dops._SUB_OPCODE_FOR_NAME[name],
        uops=lower(spec, ver="v3"),
        rd1_en=_has_src1(spec),
    )
    op = dops.DveOp(name=name, spec=spec, subdim=False,
                    uops_sha={"v3": lowered.sha("v3")})
    dops.OPS.append(op)
    dops.CUSTOM_DVE_SPECS[name] = spec
    return op


OP_SQSUM = _register("ANT_AFL_SQSUM", sq(Src0) + sq(Src1))
OP_ADDSQ = _register("ANT_AFL_ADDSQ", Src0 + sq(Src1))
OP_POLY4 = _register(
    "ANT_AFL_POLY4",
    _spill_c3_to_src1(((((Src0 * C0 + C1) * Src0 + C2) * Src0 + C3) * Src0) + One),
)
_m2 = (Src0 * C0) * Src0
OP_ACOEF = _register("ANT_AFL_ACOEF", (((_m2 + C1) * Src0 + C2) * Src0) + One)
_s2 = Src0 * Src0
OP_BCOEF = _register(
    "ANT_AFL_BCOEF", (Src0 * C1 + C2) + ((_s2 * Src0 - Src0 * Src1) * C0))
OP_DCOEF = _register(
    "ANT_AFL_DCOEF", ((Src0 * Src0) * C0 + Src1 * C1) + C2)
OP_QTH = _register("ANT_AFL_QTH", (sq(Src0) * Src1) * C0)
OP_DG2 = _register("ANT_AFL_DG2", Src0 * sq(Src1))


def _build(jpp=JPP, tiles=TILES):
    nc = bacc.Bacc("TRN2", target_bir_lowering=False, debug=False)
    xa = nc.dram_tensor("xa", (P, 7 * jpp), F32, kind="ExternalInput")
    xb = nc.dram_tensor("xb", (P, 3 * jpp), F32, kind="ExternalInput")
    ident = nc.dram_tensor("ident", (P, P), F32, kind="ExternalInput")
    ya = nc.dram_tensor("ya", (P, 3 * jpp), F32, kind="ExternalOutput")
    yb = nc.dram_tensor("yb", (P, 4 * jpp), F32, kind="ExternalOutput")
    xav = xa[:, :].rearrange("p (c j) -> p c j", j=jpp)
    xbv = xb[:, :].rearrange("p (c j) -> p c j", j=jpp)
    yav = ya[:, :].rearrange("p (c j) -> p c j", j=jpp)
    ybv = yb[:, :].rearrange("p (c j) -> p c j", j=jpp)

    mul, add, sub = OP.mult, OP.add, OP.subtract

    with TileContext(nc) as tc:
        with (
            tc.tile_pool(name="cst", bufs=1) as cstp,
            tc.tile_pool(name="io", bufs=2) as iop,
            tc.tile_pool(name="tp", bufs=1) as tp,
            tc.tile_pool(name="ps", bufs=1, space="PSUM") as psp,
        ):
            c1col = cstp.tile([P, 1], F32, name="c1col")
            nc.vector.memset(c1col, FP_C1)
            IDT = cstp.tile([P, P], F32, name="IDT")
            nc.sync.dma_start(out=IDT, in_=ident[:, :])

            off = 0
            for nf in tiles:
                INA = iop.tile([P, nf * 7], F32, tag="ina", name="tina")
                INB = iop.tile([P, nf * 3], F32, tag="inb", name="tinb")
                OUTA = iop.tile([P, nf * 3], F32, tag="outa", name="touta")
                OUTB = iop.tile([P, nf * 4], F32, tag="outb", name="toutb")
                nc.sync.dma_start(
                    out=INA.rearrange("p (c j) -> p c j", c=7),
                    in_=xav[:, :, off:off + nf])
                nc.sync.dma_start(
                    out=INB.rearrange("p (c j) -> p c j", c=3),
                    in_=xbv[:, :, off:off + nf])

                def T(nm, k=1):
                    return tp.tile([P, nf * k], F32, tag=nm, name=nm)

                def pl(t, i, k=1):
                    return t[:, i * nf:(i + k) * nf]

                def pl3(t, i=0):
                    return t[:, i * nf:(i + 3) * nf].rearrange(
                        "p (c j) -> p c j", c=3)

                def bc3(a):
                    return a.rearrange("p (o j) -> p o j", o=1).to_broadcast(
                        [P, 3, nf])

                def tt(o, a, b, op):
                    nc.vector.tensor_tensor(out=o, in0=a, in1=b, op=op)

                def stt(o, a, s, b, op0, op1):
                    nc.vector.scalar_tensor_tensor(
                        out=o, in0=a, scalar=s, in1=b, op0=op0, op1=op1)

                def cust(op_, o, a, b=None, s0=0.0, s1=0.0, imm2=0.0):
                    nc.vector._custom_dve(
                        op_, out=o, in0=a, in1=b, s0=s0, s1=s1, imm2=imm2)

                tv = pl3(INB)  # [p, 3, nf] translation planes

                u = T("u"); v = T("v")
                # e^{2s} = m00^2 + m10^2 + m20^2  (planes 6, 1, 3 of INA)
                cust(OP_SQSUM, u, pl(INA, 6), pl(INA, 1))
                e2s = T("e2s")
                cust(OP_ADDSQ, e2s, u, pl(INA, 3))
                lnd2 = T("lnd2"); es = T("es"); es2 = T("es2"); s = T("s")
                nc.scalar.activation(out=lnd2, in_=e2s, func=AF.Ln)
                nc.scalar.activation(out=es, in_=lnd2, func=AF.Exp, scale=-0.5)
                nc.scalar.activation(out=es2, in_=lnd2, func=AF.Exp, scale=-1.0)
                nc.scalar.mul(s, lnd2, 0.5)
                nc.scalar.mul(pl(OUTB, 3), lnd2, SQ3 / 2.0)   # out6

                # A3/P9 feed the PE accumulation at tile end; double-buffer
                # them so the next tile's DVE work never waits on PE drain.
                A3 = tp.tile([P, nf * 3], F32, tag="A3", name="A3", bufs=2)
                tt(pl(A3, 0), pl(INA, 0), pl(INA, 1), sub)   # a1 = m01 - m10
                tt(pl(A3, 1), pl(INA, 2), pl(INA, 3), sub)   # a2 = m02 - m20
                tt(pl(A3, 2), pl(INA, 4), pl(INA, 5), sub)   # a3 = m12 - m21
                cust(OP_SQSUM, v, pl(A3, 0), pl(A3, 1))
                S = T("S")
                cust(OP_ADDSQ, S, v, pl(A3, 2))
                # all 9 products P[i,j] = a_i * t_j at plane 3i+j
                P9 = tp.tile([P, nf * 9], F32, tag="P9", name="P9", bufs=2)
                for i in range(3):
                    tt(pl3(P9, 3 * i), bc3(pl(A3, i)), tv, mul)
                # dtil first (consumes planes 4,6,2), then ctil into 2,4,6
                dA = T("dA"); dt = T("dt")
                tt(dA, pl(P9, 4), pl(P9, 6), sub)               # a2t1-a3t0
                tt(dt, dA, pl(P9, 2), sub)                      # - a1t2
                tt(pl(P9, 2), pl(P9, 1), pl(P9, 5), add)        # cx
                tt(pl(P9, 4), pl(P9, 8), pl(P9, 0), sub)        # cy
                stt(pl(P9, 6), pl(P9, 7), -1.0, pl(P9, 3), mul, sub)  # cz
                # scalar chain (ACT outputs ready by now)
                z = T("z")
                tt(z, es2, S, mul)                  # z = 4 sin^2 th
                fp = T("fp")
                cust(OP_POLY4, fp, z, c1col, s0=FP_C4, s1=FP_C3, imm2=FP_C2)
                qt = T("qt")
                cust(OP_QTH, qt, fp, z, s0=0.25)    # th^2
                g = T("g")
                stt(g, fp, 0.5, es, mul, mul)       # g = f e^{-s}
                # rotation outputs = sqrt2 * g * a_k -> OUTB planes 0..2
                stt(pl3(OUTB), bc3(g), SQ2, pl3(A3), mul, mul)
                nc.sync.dma_start(
                    out=ybv[:, :, off:off + nf],
                    in_=OUTB.rearrange("p (c j) -> p c j", c=4))
                # psi coefficients (slots reuse dead temps)
                A = T("e2s"); Bc = T("S"); D = T("lnd2")
                cust(OP_ACOEF, A, s,
                     s0=-1.0 / 720.0, s1=1.0 / 12.0, imm2=-0.5)
                cust(OP_BCOEF, Bc, s, qt,
                     s0=-1.0 / 180.0, s1=1.0 / 6.0, imm2=-0.5)
                cust(OP_DCOEF, D, s, qt,
                     s0=-1.0 / 120.0, s1=1.0 / 720.0, imm2=1.0 / 12.0)
                v2 = T("u"); Ap = T("es2"); Bg = T("s"); Dg2 = T("fp")
                tt(v2, D, qt, mul)
                tt(Ap, A, v2, sub)
                # w1 product as early as possible so PE can start its
                # accumulation while the DVE computes w2/pw.
                W1 = T("W1", 3)
                tt(pl3(W1), bc3(Ap), tv, mul)
                tt(Bg, Bc, g, mul)
                cust(OP_DG2, Dg2, D, g)
                P3 = T("z")
                tt(P3, Dg2, dt, mul)
                # pw = P3*(-a3,+a2,-a1) into free P9 planes 1,3,5
                stt(pl(P9, 1), P3, -1.0, pl(A3, 2), mul, mul)
                tt(pl(P9, 3), P3, pl(A3, 1), mul)
                stt(pl(P9, 5), P3, -1.0, pl(A3, 0), mul, mul)
                # w2 = Bg*ctil (into A3, fully consumed)
                cview = P9[:, 2 * nf:8 * nf].rearrange(
                    "p (c t j) -> p c t j", c=3, t=2)[:, :, 0, :]
                pwview = P9[:, 1 * nf:7 * nf].rearrange(
                    "p (c t j) -> p c t j", c=3, t=2)[:, :, 0, :]
                tt(pl3(A3), bc3(Bg), cview, mul)
                # sum w1 + w2 + pw on the (idle) TensorEngine: identity-weight
                # matmuls accumulate the three terms per plane in PSUM, ACT
                # copies PSUM -> SBUF. Frees 6 DVE units per tile.
                for i in range(3):
                    PSi = psp.tile([P, nf], F32, tag=f"ps{i}", name=f"ps{i}")
                    pwsrc = pl(P9, 1 + 2 * i)
                    terms = (pl(W1, i), pl(A3, i), pwsrc)
                    for c0 in range(0, nf, 512):
                        w = min(512, nf - c0)
                        for k, src in enumerate(terms):
                            nc.tensor.matmul(
                                PSi[:, c0:c0 + w],
                                IDT[:, :], src[:, c0:c0 + w],
                                start=(k == 0), stop=(k == 2))
                    nc.scalar.copy(pl(OUTA, i), PSi[:, :])
                    nc.sync.dma_start(
                        out=yav[:, i, off:off + nf], in_=pl(OUTA, i))
                off += nf
    if not nc.is_finalized():
        nc.finalize()
    return nc


def _pack(affine):
    """(B,4,4) f32 -> per-core channel-planar arrays xa (P,7*jpp), xb (P,3*jpp)."""
    x = np.ascontiguousarray(affine.reshape(B, 16).astype(np.float32, copy=False))
    pad = NCORES * NC_ELEMS - B
    padblk = np.zeros((pad, 16), np.float32)
    padblk[:, [0, 5, 10, 15]] = 1.0  # identity affines -> log = 0
    data = np.concatenate([x, padblk], 0).reshape(NCORES, P, JPP, 16)
    da = np.ascontiguousarray(data[:, :, :, CH_A].transpose(0, 1, 3, 2))
    db = np.ascontiguousarray(data[:, :, :, CH_B].transpose(0, 1, 3, 2))
    return (da.reshape(NCORES, P, 7 * JPP), db.reshape(NCORES, P, 3 * JPP))


def _run(affine, trace=False):
    da, db = _pack(np.asarray(affine))
    nc = _build()
    eye = np.ascontiguousarray(np.eye(P, dtype=np.float32))
    res = run_bass_kernel_spmd(
        nc,
        [{"xa": da[i], "xb": db[i], "ident": eye} for i in range(NCORES)],
        core_ids=list(range(NCORES)),
        trace=trace,
    )
    out = np.empty((NCORES, P, JPP, 7), np.float32)
    for i, r in enumerate(res.results):
        out[i, :, :, 0:3] = r["ya"].reshape(P, 3, JPP).transpose(0, 2, 1)
        out[i, :, :, 3:7] = r["yb"].reshape(P, 4, JPP).transpose(0, 2, 1)
    return out.reshape(NCORES * NC_ELEMS, 7)[:B], res


def kernel(affine):
    y, _ = _run(np.asarray(affine), trace=False)
    return y


# revision 33
# speedup vs baseline: 1.3641x; 1.0068x over previous
"""nn_AffineLog: batched 4x4 affine matrix-log projected onto the 7-dim CSO basis.

Closed-form algorithm (replaces the reference's 24-term Mercator series):
inputs are exactly [[e^s R, t],[0,1]] with R a rotation, so
  L3x3 = s I + f (R - R^T),  f = asin(x)/(2x), x = sin th  (poly in x^2)
  translation u' = psi(C) t, psi(x) = x/(e^x-1), reduced via
  Omega^3 = -th^2 Omega to u' = (A - D q) t + B (w x t) + D (w.t) w.

Data-parallel over 8 NeuronCores. The host packs the 10 live channels of
each affine into channel-planar per-partition DRAM, so every DVE access is
contiguous; elementwise pipeline on DVE/ACT with custom fused DVE ops.
"""

import os

os.environ.setdefault("BY_DEFAULT_DISABLE_SUBTILE_DEPS", "1")

import functools
from contextlib import ExitStack

import numpy as np

import concourse.bass as bass
import concourse.bacc as bacc
import concourse.hw_specs as hw_specs
import concourse.mybir as mybir
from concourse.tile import TileContext
from concourse.bass_utils import run_bass_kernel_spmd
from concourse import dve_ops as dops
from concourse.dve_spec import (
    Spec, Src0, Src1, C0, C1, C2, C3, One, sq, _spill_c3_to_src1, lower,
    _has_src1,
)
from concourse.dve_uop import DveOpSpec

AF = mybir.ActivationFunctionType
OP = mybir.AluOpType
F32 = mybir.dt.float32

NCORES = 8
B = 2_000_000
P = 128
JPP = 1954                  # free-dim elements per partition per core
NC_ELEMS = P * JPP          # 250112 per core (total 2000896, pad 896)
TILES = (432, 648, 648, 226)

# packed channel order (host): [m01, m10, m02, m20, m12, m21, m00] + [t0, t1, t2]
CH_A = [1, 4, 2, 8, 6, 9, 0]   # 7 "matrix" planes -> tensor xa
CH_B = [3, 7, 11]              # 3 translation planes -> tensor xb

SQ2 = float(np.sqrt(2.0))
SQ3 = float(np.sqrt(3.0))
# f'(z) = 2*asin(x)/(2x) with z = 4x^2:  f' = 1 + c1 z + c2 z^2 + c3 z^3 + c4 z^4
FP_C1 = 1.0 / 24.0
FP_C2 = 2.0 * 0.5 * (3.0 / 40.0) / 16.0
FP_C3 = 2.0 * 0.5 * (5.0 / 112.0) / 64.0
FP_C4 = 2.0 * 0.5 * (35.0 / 1152.0) / 256.0

# Restrict ACT table choice to the one set holding ln+exp+copy, so bacc
# never alternates table loads between tiles. Other set names stay (ids are
# positional) but advertise no functions.
_orig_gat = hw_specs.get_activation_tables


@functools.cache
def _gat_ln_exp_only(module_arch):
    t = _orig_gat(module_arch)
    keep = "natural_log_exp_and_others"
    return {k: (v if k == keep else set()) for k, v in t.items()}


hw_specs.get_activation_tables = _gat_ln_exp_only
bacc.get_activation_tables = _gat_ln_exp_only


# --- custom fused DVE ops (registered into concourse.dve_ops at import) ----
def _register(name, body):
    if name in dops._SUB_OPCODE_FOR_NAME:
        return next(o for o in dops.OPS if o.name == name)
    dops._SUB_OPCODE_FOR_NAME[name] = dops._CUSTOM_DVE_ROW_BASE + len(dops.OPS)
    assert dops._SUB_OPCODE_FOR_NAME[name] < 0x20
    spec = Spec(body=body)
    lowered = DveOpSpec(
        name=name,
        opcode=dops._SUB_OPCODE_FOR_NAME[name],
        uops=lower(spec, ver="v3"),
        rd1_en=_has_src1(spec),
    )
    op = dops.DveOp(name=name, spec=spec, subdim=False,
                    uops_sha={"v3": lowered.sha("v3")})
    dops.OPS.append(op)
    dops.CUSTOM_DVE_SPECS[name] = spec
    return op


OP_SQSUM = _register("ANT_AFL_SQSUM", sq(Src0) + sq(Src1))
OP_ADDSQ = _register("ANT_AFL_ADDSQ", Src0 + sq(Src1))
OP_POLY4 = _register(
    "ANT_AFL_POLY4",
    _spill_c3_to_src1(((((Src0 * C0 + C1) * Src0 + C2) * Src0 + C3) * Src0) + One),
)
_m2 = (Src0 * C0) * Src0
# Ap = A(s) - Src1  (Src1 = D*qt folded in, saving a separate subtract)
OP_APCOEF = _register(
    "ANT_AFL_APCOEF", ((((_m2 + C1) * Src0 + C2) * Src0) + One) - Src1)
_s2 = Src0 * Src0
OP_BCOEF = _register(
    "ANT_AFL_BCOEF", (Src0 * C1 + C2) + ((_s2 * Src0 - Src0 * Src1) * C0))
OP_DCOEF = _register(
    "ANT_AFL_DCOEF", ((Src0 * Src0) * C0 + Src1 * C1) + C2)
OP_QTH = _register("ANT_AFL_QTH", (sq(Src0) * Src1) * C0)
OP_DG2 = _register("ANT_AFL_DG2", Src0 * sq(Src1))


def _build(jpp=JPP, tiles=TILES):
    nc = bacc.Bacc("TRN2", target_bir_lowering=False, debug=False)
    xa = nc.dram_tensor("xa", (P, 7 * jpp), F32, kind="ExternalInput")
    xb = nc.dram_tensor("xb", (P, 3 * jpp), F32, kind="ExternalInput")
    ident = nc.dram_tensor("ident", (P, P), F32, kind="ExternalInput")
    ya = nc.dram_tensor("ya", (P, 3 * jpp), F32, kind="ExternalOutput")
    yb = nc.dram_tensor("yb", (P, 4 * jpp), F32, kind="ExternalOutput")
    xav = xa[:, :].rearrange("p (c j) -> p c j", j=jpp)
    xbv = xb[:, :].rearrange("p (c j) -> p c j", j=jpp)
    yav = ya[:, :].rearrange("p (c j) -> p c j", j=jpp)
    ybv = yb[:, :].rearrange("p (c j) -> p c j", j=jpp)

    mul, add, sub = OP.mult, OP.add, OP.subtract

    with TileContext(nc) as tc:
        with (
            tc.tile_pool(name="cst", bufs=1) as cstp,
            tc.tile_pool(name="io", bufs=2) as iop,
            tc.tile_pool(name="tp", bufs=1) as tp,
            tc.tile_pool(name="ps", bufs=1, space="PSUM") as psp,
        ):
            c1col = cstp.tile([P, 1], F32, name="c1col")
            nc.vector.memset(c1col, FP_C1)
            IDT = cstp.tile([P, P], F32, name="IDT")
            nc.sync.dma_start(out=IDT, in_=ident[:, :])

            off = 0
            for nf in tiles:
                INA = iop.tile([P, nf * 7], F32, tag="ina", name="tina")
                INB = iop.tile([P, nf * 3], F32, tag="inb", name="tinb")
                OUTA = iop.tile([P, nf * 3], F32, tag="outa", name="touta")
                OUTB = iop.tile([P, nf * 4], F32, tag="outb", name="toutb")
                nc.sync.dma_start(
                    out=INA.rearrange("p (c j) -> p c j", c=7),
                    in_=xav[:, :, off:off + nf])
                nc.sync.dma_start(
                    out=INB.rearrange("p (c j) -> p c j", c=3),
                    in_=xbv[:, :, off:off + nf])

                def T(nm, k=1):
                    return tp.tile([P, nf * k], F32, tag=nm, name=nm)

                def pl(t, i, k=1):
                    return t[:, i * nf:(i + k) * nf]

                def pl3(t, i=0):
                    return t[:, i * nf:(i + 3) * nf].rearrange(
                        "p (c j) -> p c j", c=3)

                def bc3(a):
                    return a.rearrange("p (o j) -> p o j", o=1).to_broadcast(
                        [P, 3, nf])

                def tt(o, a, b, op):
                    nc.vector.tensor_tensor(out=o, in0=a, in1=b, op=op)

                def stt(o, a, s, b, op0, op1):
                    nc.vector.scalar_tensor_tensor(
                        out=o, in0=a, scalar=s, in1=b, op0=op0, op1=op1)

                def cust(op_, o, a, b=None, s0=0.0, s1=0.0, imm2=0.0):
                    nc.vector._custom_dve(
                        op_, out=o, in0=a, in1=b, s0=s0, s1=s1, imm2=imm2)

                tv = pl3(INB)  # [p, 3, nf] translation planes

                u = T("u"); v = T("v")
                # e^{2s} = m00^2 + m10^2 + m20^2  (planes 6, 1, 3 of INA)
                cust(OP_SQSUM, u, pl(INA, 6), pl(INA, 1))
                e2s = T("e2s")
                cust(OP_ADDSQ, e2s, u, pl(INA, 3))
                lnd2 = T("lnd2"); es = T("es"); es2 = T("es2"); s = T("s")
                nc.scalar.activation(out=lnd2, in_=e2s, func=AF.Ln)
                nc.scalar.activation(out=es, in_=lnd2, func=AF.Exp, scale=-0.5)
                nc.scalar.activation(out=es2, in_=lnd2, func=AF.Exp, scale=-1.0)
                nc.scalar.mul(s, lnd2, 0.5)
                nc.scalar.mul(pl(OUTB, 3), lnd2, SQ3 / 2.0)   # out6

                # A3/P9 feed the PE accumulation at tile end; double-buffer
                # them so the next tile's DVE work never waits on PE drain.
                A3 = tp.tile([P, nf * 3], F32, tag="A3", name="A3", bufs=2)
                tt(pl(A3, 0), pl(INA, 0), pl(INA, 1), sub)   # a1 = m01 - m10
                tt(pl(A3, 1), pl(INA, 2), pl(INA, 3), sub)   # a2 = m02 - m20
                tt(pl(A3, 2), pl(INA, 4), pl(INA, 5), sub)   # a3 = m12 - m21
                cust(OP_SQSUM, v, pl(A3, 0), pl(A3, 1))
                S = T("S")
                cust(OP_ADDSQ, S, v, pl(A3, 2))
                # all 9 products P[i,j] = a_i * t_j at plane 3i+j
                P9 = tp.tile([P, nf * 9], F32, tag="P9", name="P9", bufs=2)
                for i in range(3):
                    tt(pl3(P9, 3 * i), bc3(pl(A3, i)), tv, mul)
                # dtil first (consumes planes 4,6,2), then ctil into 2,4,6
                dA = T("dA"); dt = T("dt")
                tt(dA, pl(P9, 4), pl(P9, 6), sub)               # a2t1-a3t0
                tt(dt, dA, pl(P9, 2), sub)                      # - a1t2
                tt(pl(P9, 2), pl(P9, 1), pl(P9, 5), add)        # cx
                tt(pl(P9, 4), pl(P9, 8), pl(P9, 0), sub)        # cy
                stt(pl(P9, 6), pl(P9, 7), -1.0, pl(P9, 3), mul, sub)  # cz
                # scalar chain (ACT outputs ready by now)
                z = T("z")
                tt(z, es2, S, mul)                  # z = 4 sin^2 th
                fp = T("fp")
                cust(OP_POLY4, fp, z, c1col, s0=FP_C4, s1=FP_C3, imm2=FP_C2)
                qt = T("qt")
                cust(OP_QTH, qt, fp, z, s0=0.25)    # th^2
                g = T("g")
                stt(g, fp, 0.5, es, mul, mul)       # g = f e^{-s}
                # rotation outputs = sqrt2 * g * a_k -> OUTB planes 0..2
                stt(pl3(OUTB), bc3(g), SQ2, pl3(A3), mul, mul)
                nc.sync.dma_start(
                    out=ybv[:, :, off:off + nf],
                    in_=OUTB.rearrange("p (c j) -> p c j", c=4))
                # psi coefficients (slots reuse dead temps)
                Bc = T("S"); D = T("lnd2")
                cust(OP_BCOEF, Bc, s, qt,
                     s0=-1.0 / 180.0, s1=1.0 / 6.0, imm2=-0.5)
                cust(OP_DCOEF, D, s, qt,
                     s0=-1.0 / 120.0, s1=1.0 / 720.0, imm2=1.0 / 12.0)
                v2 = T("u"); Ap = T("es2"); Bg = T("s"); Dg2 = T("fp")
                tt(v2, D, qt, mul)
                cust(OP_APCOEF, Ap, s, v2,
                     s0=-1.0 / 720.0, s1=1.0 / 12.0, imm2=-0.5)
                # w1 product as early as possible so PE can start its
                # accumulation while the DVE computes w2/pw.
                W1 = T("W1", 3)
                tt(pl3(W1), bc3(Ap), tv, mul)
                tt(Bg, Bc, g, mul)
                cust(OP_DG2, Dg2, D, g)
                P3 = T("z")
                tt(P3, Dg2, dt, mul)
                # pw = P3*(-a3,+a2,-a1) into free P9 planes 1,3,5
                stt(pl(P9, 1), P3, -1.0, pl(A3, 2), mul, mul)
                tt(pl(P9, 3), P3, pl(A3, 1), mul)
                stt(pl(P9, 5), P3, -1.0, pl(A3, 0), mul, mul)
                # w2 = Bg*ctil (into A3, fully consumed)
                cview = P9[:, 2 * nf:8 * nf].rearrange(
                    "p (c t j) -> p c t j", c=3, t=2)[:, :, 0, :]
                pwview = P9[:, 1 * nf:7 * nf].rearrange(
                    "p (c t j) -> p c t j", c=3, t=2)[:, :, 0, :]
                tt(pl3(A3), bc3(Bg), cview, mul)
                # sum w1 + w2 + pw on the (idle) TensorEngine: identity-weight
                # matmuls accumulate the three terms per plane in PSUM, ACT
                # copies PSUM -> SBUF. Frees 6 DVE units per tile.
                for i in range(3):
                    PSi = psp.tile([P, nf], F32, tag=f"ps{i}", name=f"ps{i}")
                    pwsrc = pl(P9, 1 + 2 * i)
                    terms = (pl(W1, i), pl(A3, i), pwsrc)
                    for c0 in range(0, nf, 512):
                        w = min(512, nf - c0)
                        for k, src in enumerate(terms):
                            nc.tensor.matmul(
                                PSi[:, c0:c0 + w],
                                IDT[:, :], src[:, c0:c0 + w],
                                start=(k == 0), stop=(k == 2))
                    nc.scalar.copy(pl(OUTA, i), PSi[:, :])
                    nc.sync.dma_start(
                        out=yav[:, i, off:off + nf], in_=pl(OUTA, i))
                off += nf
    if not nc.is_finalized():
        nc.finalize()
    return nc


def _pack(affine):
    """(B,4,4) f32 -> per-core channel-planar arrays xa (P,7*jpp), xb (P,3*jpp)."""
    x = np.ascontiguousarray(affine.reshape(B, 16).astype(np.float32, copy=False))
    pad = NCORES * NC_ELEMS - B
    padblk = np.zeros((pad, 16), np.float32)
    padblk[:, [0, 5, 10, 15]] = 1.0  # identity affines -> log = 0
    data = np.concatenate([x, padblk], 0).reshape(NCORES, P, JPP, 16)
    da = np.ascontiguousarray(data[:, :, :, CH_A].transpose(0, 1, 3, 2))
    db = np.ascontiguousarray(data[:, :, :, CH_B].transpose(0, 1, 3, 2))
    return (da.reshape(NCORES, P, 7 * JPP), db.reshape(NCORES, P, 3 * JPP))


def _run(affine, trace=False):
    da, db = _pack(np.asarray(affine))
    nc = _build()
    eye = np.ascontiguousarray(np.eye(P, dtype=np.float32))
    res = run_bass_kernel_spmd(
        nc,
        [{"xa": da[i], "xb": db[i], "ident": eye} for i in range(NCORES)],
        core_ids=list(range(NCORES)),
        trace=trace,
    )
    out = np.empty((NCORES, P, JPP, 7), np.float32)
    for i, r in enumerate(res.results):
        out[i, :, :, 0:3] = r["ya"].reshape(P, 3, JPP).transpose(0, 2, 1)
        out[i, :, :, 3:7] = r["yb"].reshape(P, 4, JPP).transpose(0, 2, 1)
    return out.reshape(NCORES * NC_ELEMS, 7)[:B], res


def kernel(affine):
    y, _ = _run(np.asarray(affine), trace=False)
    return y


# revision 34
# speedup vs baseline: 1.4128x; 1.0357x over previous
"""nn_AffineLog: batched 4x4 affine matrix-log projected onto the 7-dim CSO basis.

Closed-form algorithm (replaces the reference's 24-term Mercator series):
inputs are exactly [[e^s R, t],[0,1]] with R a rotation, so
  L3x3 = s I + f (R - R^T),  f = asin(x)/(2x), x = sin th  (poly in x^2)
  translation u' = psi(C) t, psi(x) = x/(e^x-1), reduced via
  Omega^3 = -th^2 Omega to u' = (A - D q) t + B (w x t) + D (w.t) w.

Data-parallel over 8 NeuronCores. The host packs the 10 live channels of
each affine into channel-planar per-partition DRAM, so every DVE access is
contiguous; elementwise pipeline on DVE/ACT with custom fused DVE ops.
"""

import os

os.environ.setdefault("BY_DEFAULT_DISABLE_SUBTILE_DEPS", "1")

import functools
from contextlib import ExitStack

import numpy as np

import concourse.bass as bass
import concourse.bacc as bacc
import concourse.hw_specs as hw_specs
import concourse.mybir as mybir
from concourse.tile import TileContext
from concourse.bass_utils import run_bass_kernel_spmd
from concourse import dve_ops as dops
from concourse.dve_spec import (
    Spec, Src0, Src1, C0, C1, C2, C3, One, sq, _spill_c3_to_src1, lower,
    _has_src1,
)
from concourse.dve_uop import DveOpSpec

AF = mybir.ActivationFunctionType
OP = mybir.AluOpType
F32 = mybir.dt.float32

NCORES = 8
B = 2_000_000
P = 128
JPP = 1954                  # free-dim elements per partition per core
NC_ELEMS = P * JPP          # 250112 per core (total 2000896, pad 896)
TILES = (432, 648, 648, 226)

# packed channel order (host): [m01, m10, m02, m20, m12, m21, m00] + [t0, t1, t2]
CH_A = [1, 4, 2, 8, 6, 9, 0]   # 7 "matrix" planes -> tensor xa
CH_B = [3, 7, 11]              # 3 translation planes -> tensor xb

SQ2 = float(np.sqrt(2.0))
SQ3 = float(np.sqrt(3.0))
# f'(z) = 2*asin(x)/(2x) with z = 4x^2:  f' = 1 + c1 z + c2 z^2 + c3 z^3 + c4 z^4
FP_C1 = 1.0 / 24.0
FP_C2 = 2.0 * 0.5 * (3.0 / 40.0) / 16.0
FP_C3 = 2.0 * 0.5 * (5.0 / 112.0) / 64.0
FP_C4 = 2.0 * 0.5 * (35.0 / 1152.0) / 256.0

# Restrict ACT table choice to the one set holding ln+exp+copy, so bacc
# never alternates table loads between tiles. Other set names stay (ids are
# positional) but advertise no functions.
_orig_gat = hw_specs.get_activation_tables


@functools.cache
def _gat_ln_exp_only(module_arch):
    t = _orig_gat(module_arch)
    keep = "natural_log_exp_and_others"
    return {k: (v if k == keep else set()) for k, v in t.items()}


hw_specs.get_activation_tables = _gat_ln_exp_only
bacc.get_activation_tables = _gat_ln_exp_only


# --- custom fused DVE ops (registered into concourse.dve_ops at import) ----
def _register(name, body):
    if name in dops._SUB_OPCODE_FOR_NAME:
        return next(o for o in dops.OPS if o.name == name)
    dops._SUB_OPCODE_FOR_NAME[name] = dops._CUSTOM_DVE_ROW_BASE + len(dops.OPS)
    assert dops._SUB_OPCODE_FOR_NAME[name] < 0x20
    spec = Spec(body=body)
    lowered = DveOpSpec(
        name=name,
        opcode=dops._SUB_OPCODE_FOR_NAME[name],
        uops=lower(spec, ver="v3"),
        rd1_en=_has_src1(spec),
    )
    op = dops.DveOp(name=name, spec=spec, subdim=False,
                    uops_sha={"v3": lowered.sha("v3")})
    dops.OPS.append(op)
    dops.CUSTOM_DVE_SPECS[name] = spec
    return op


OP_SQSUM = _register("ANT_AFL_SQSUM", sq(Src0) + sq(Src1))
OP_ADDSQ = _register("ANT_AFL_ADDSQ", Src0 + sq(Src1))
OP_POLY4 = _register(
    "ANT_AFL_POLY4",
    _spill_c3_to_src1(((((Src0 * C0 + C1) * Src0 + C2) * Src0 + C3) * Src0) + One),
)
_m2 = (Src0 * C0) * Src0
# Ap = A(s) - Src1  (Src1 = D*qt folded in, saving a separate subtract)
OP_APCOEF = _register(
    "ANT_AFL_APCOEF", ((((_m2 + C1) * Src0 + C2) * Src0) + One) - Src1)
_s2 = Src0 * Src0
OP_BCOEF = _register(
    "ANT_AFL_BCOEF", (Src0 * C1 + C2) + ((_s2 * Src0 - Src0 * Src1) * C0))
OP_DCOEF = _register(
    "ANT_AFL_DCOEF", ((Src0 * Src0) * C0 + Src1 * C1) + C2)
OP_QTH = _register("ANT_AFL_QTH", (sq(Src0) * Src1) * C0)
OP_DG2 = _register("ANT_AFL_DG2", Src0 * sq(Src1))


def _build(jpp=JPP, tiles=TILES):
    nc = bacc.Bacc("TRN2", target_bir_lowering=False, debug=False)
    xa = nc.dram_tensor("xa", (P, 7 * jpp), F32, kind="ExternalInput")
    xb = nc.dram_tensor("xb", (P, 3 * jpp), F32, kind="ExternalInput")
    ident = nc.dram_tensor("ident", (P, P), F32, kind="ExternalInput")
    ya = nc.dram_tensor("ya", (P, 3 * jpp), F32, kind="ExternalOutput")
    yb = nc.dram_tensor("yb", (P, 4 * jpp), F32, kind="ExternalOutput")
    xav = xa[:, :].rearrange("p (c j) -> p c j", j=jpp)
    xbv = xb[:, :].rearrange("p (c j) -> p c j", j=jpp)
    yav = ya[:, :].rearrange("p (c j) -> p c j", j=jpp)
    ybv = yb[:, :].rearrange("p (c j) -> p c j", j=jpp)

    mul, add, sub = OP.mult, OP.add, OP.subtract

    with TileContext(nc) as tc:
        with (
            tc.tile_pool(name="cst", bufs=1) as cstp,
            tc.tile_pool(name="io", bufs=2) as iop,
            tc.tile_pool(name="tp", bufs=1) as tp,
            tc.tile_pool(name="ps", bufs=1, space="PSUM") as psp,
        ):
            c1col = cstp.tile([P, 1], F32, name="c1col")
            nc.vector.memset(c1col, FP_C1)
            IDT = cstp.tile([P, P], F32, name="IDT")
            nc.sync.dma_start(out=IDT, in_=ident[:, :])

            off = 0
            for nf in tiles:
                INA = iop.tile([P, nf * 7], F32, tag="ina", name="tina")
                INB = iop.tile([P, nf * 3], F32, tag="inb", name="tinb")
                OUTA = iop.tile([P, nf * 3], F32, tag="outa", name="touta")
                OUTB = iop.tile([P, nf * 4], F32, tag="outb", name="toutb")
                nc.sync.dma_start(
                    out=INA.rearrange("p (c j) -> p c j", c=7),
                    in_=xav[:, :, off:off + nf])
                nc.sync.dma_start(
                    out=INB.rearrange("p (c j) -> p c j", c=3),
                    in_=xbv[:, :, off:off + nf])

                def T(nm, k=1):
                    return tp.tile([P, nf * k], F32, tag=nm, name=nm)

                def pl(t, i, k=1):
                    return t[:, i * nf:(i + k) * nf]

                def pl3(t, i=0):
                    return t[:, i * nf:(i + 3) * nf].rearrange(
                        "p (c j) -> p c j", c=3)

                def bc3(a):
                    return a.rearrange("p (o j) -> p o j", o=1).to_broadcast(
                        [P, 3, nf])

                def tt(o, a, b, op):
                    nc.vector.tensor_tensor(out=o, in0=a, in1=b, op=op)

                def stt(o, a, s, b, op0, op1):
                    nc.vector.scalar_tensor_tensor(
                        out=o, in0=a, scalar=s, in1=b, op0=op0, op1=op1)

                def cust(op_, o, a, b=None, s0=0.0, s1=0.0, imm2=0.0):
                    nc.vector._custom_dve(
                        op_, out=o, in0=a, in1=b, s0=s0, s1=s1, imm2=imm2)

                tv = pl3(INB)  # [p, 3, nf] translation planes

                u = T("u"); v = T("v")
                # e^{2s} = m00^2 + m10^2 + m20^2  (planes 6, 1, 3 of INA)
                cust(OP_SQSUM, u, pl(INA, 6), pl(INA, 1))
                e2s = T("e2s")
                cust(OP_ADDSQ, e2s, u, pl(INA, 3))
                lnd2 = T("lnd2"); es = T("es"); es2 = T("es2"); s = T("s")
                nc.scalar.activation(out=lnd2, in_=e2s, func=AF.Ln)
                nc.scalar.activation(out=es, in_=lnd2, func=AF.Exp, scale=-0.5)
                nc.scalar.activation(out=es2, in_=lnd2, func=AF.Exp, scale=-1.0)
                nc.scalar.mul(s, lnd2, 0.5)
                nc.scalar.mul(pl(OUTB, 3), lnd2, SQ3 / 2.0)   # out6

                # A3/P9 feed the PE accumulation at tile end; double-buffer
                # them so the next tile's DVE work never waits on PE drain.
                A3 = tp.tile([P, nf * 3], F32, tag="A3", name="A3", bufs=2)
                tt(pl(A3, 0), pl(INA, 0), pl(INA, 1), sub)   # a1 = m01 - m10
                tt(pl(A3, 1), pl(INA, 2), pl(INA, 3), sub)   # a2 = m02 - m20
                tt(pl(A3, 2), pl(INA, 4), pl(INA, 5), sub)   # a3 = m12 - m21
                cust(OP_SQSUM, v, pl(A3, 0), pl(A3, 1))
                S = T("S")
                cust(OP_ADDSQ, S, v, pl(A3, 2))
                # all 9 products P[i,j] = a_i * t_j at plane 3i+j
                P9 = tp.tile([P, nf * 9], F32, tag="P9", name="P9", bufs=2)
                for i in range(3):
                    tt(pl3(P9, 3 * i), bc3(pl(A3, i)), tv, mul)
                # dtil first (consumes planes 4,6,2), then ctil into 2,4,6
                dA = T("dA"); dt = T("dt")
                tt(dA, pl(P9, 4), pl(P9, 6), sub)               # a2t1-a3t0
                tt(dt, dA, pl(P9, 2), sub)                      # - a1t2
                tt(pl(P9, 2), pl(P9, 1), pl(P9, 5), add)        # cx
                tt(pl(P9, 4), pl(P9, 8), pl(P9, 0), sub)        # cy
                stt(pl(P9, 6), pl(P9, 7), -1.0, pl(P9, 3), mul, sub)  # cz
                # scalar chain (ACT outputs ready by now)
                z = T("z")
                tt(z, es2, S, mul)                  # z = 4 sin^2 th
                fp = T("fp")
                cust(OP_POLY4, fp, z, c1col, s0=FP_C4, s1=FP_C3, imm2=FP_C2)
                qt = T("qt")
                cust(OP_QTH, qt, fp, z, s0=0.25)    # th^2
                g = T("g")
                stt(g, fp, 0.5, es, mul, mul)       # g = f e^{-s}
                # rotation outputs = sqrt2 * g * a_k -> OUTB planes 0..2
                stt(pl3(OUTB), bc3(g), SQ2, pl3(A3), mul, mul)
                nc.sync.dma_start(
                    out=ybv[:, :, off:off + nf],
                    in_=OUTB.rearrange("p (c j) -> p c j", c=4))
                # psi coefficients (slots reuse dead temps)
                Bc = T("S"); D = T("lnd2")
                cust(OP_BCOEF, Bc, s, qt,
                     s0=-1.0 / 180.0, s1=1.0 / 6.0, imm2=-0.5)
                cust(OP_DCOEF, D, s, qt,
                     s0=-1.0 / 120.0, s1=1.0 / 720.0, imm2=1.0 / 12.0)
                v2 = T("u"); Ap = T("es2"); Bg = T("s"); Dg2 = T("fp")
                tt(v2, D, qt, mul)
                cust(OP_APCOEF, Ap, s, v2,
                     s0=-1.0 / 720.0, s1=1.0 / 12.0, imm2=-0.5)
                # w1 product as early as possible so PE can start its
                # accumulation while the DVE computes w2/pw.
                W1 = T("W1", 3)
                tt(pl3(W1), bc3(Ap), tv, mul)
                tt(Bg, Bc, g, mul)
                cust(OP_DG2, Dg2, D, g)
                P3 = T("z")
                tt(P3, Dg2, dt, mul)
                # pw = P3*(-a3,+a2,-a1) into free P9 planes 1,3,5
                stt(pl(P9, 1), P3, -1.0, pl(A3, 2), mul, mul)
                tt(pl(P9, 3), P3, pl(A3, 1), mul)
                stt(pl(P9, 5), P3, -1.0, pl(A3, 0), mul, mul)
                # w2 = Bg*ctil (into A3, fully consumed)
                cview = P9[:, 2 * nf:8 * nf].rearrange(
                    "p (c t j) -> p c t j", c=3, t=2)[:, :, 0, :]
                pwview = P9[:, 1 * nf:7 * nf].rearrange(
                    "p (c t j) -> p c t j", c=3, t=2)[:, :, 0, :]
                tt(pl3(A3), bc3(Bg), cview, mul)
                # sum w1 + w2 + pw. Large tiles: identity-weight matmuls
                # accumulate the three terms per plane in PSUM (idle PE), ACT
                # copies PSUM -> SBUF — frees 6 DVE units and overlaps the
                # next tile. Small (last) tile: plain DVE adds — the serial
                # PE chain would lengthen the kernel tail.
                if nf > 300:
                    for i in range(3):
                        PSi = psp.tile([P, nf], F32, tag=f"ps{i}", name=f"ps{i}")
                        pwsrc = pl(P9, 1 + 2 * i)
                        terms = (pl(W1, i), pl(A3, i), pwsrc)
                        for c0 in range(0, nf, 512):
                            w = min(512, nf - c0)
                            for k, src in enumerate(terms):
                                nc.tensor.matmul(
                                    PSi[:, c0:c0 + w],
                                    IDT[:, :], src[:, c0:c0 + w],
                                    start=(k == 0), stop=(k == 2))
                        nc.scalar.copy(pl(OUTA, i), PSi[:, :])
                        nc.sync.dma_start(
                            out=yav[:, i, off:off + nf], in_=pl(OUTA, i))
                else:
                    tt(pl3(W1), pl3(W1), pl3(A3), add)
                    tt(pl3(OUTA), pl3(W1), pwview, add)
                    nc.sync.dma_start(
                        out=yav[:, :, off:off + nf],
                        in_=OUTA.rearrange("p (c j) -> p c j", c=3))
                off += nf
    if not nc.is_finalized():
        nc.finalize()
    return nc


def _pack(affine):
    """(B,4,4) f32 -> per-core channel-planar arrays xa (P,7*jpp), xb (P,3*jpp)."""
    x = np.ascontiguousarray(affine.reshape(B, 16).astype(np.float32, copy=False))
    pad = NCORES * NC_ELEMS - B
    padblk = np.zeros((pad, 16), np.float32)
    padblk[:, [0, 5, 10, 15]] = 1.0  # identity affines -> log = 0
    data = np.concatenate([x, padblk], 0).reshape(NCORES, P, JPP, 16)
    da = np.ascontiguousarray(data[:, :, :, CH_A].transpose(0, 1, 3, 2))
    db = np.ascontiguousarray(data[:, :, :, CH_B].transpose(0, 1, 3, 2))
    return (da.reshape(NCORES, P, 7 * JPP), db.reshape(NCORES, P, 3 * JPP))


def _run(affine, trace=False):
    da, db = _pack(np.asarray(affine))
    nc = _build()
    eye = np.ascontiguousarray(np.eye(P, dtype=np.float32))
    res = run_bass_kernel_spmd(
        nc,
        [{"xa": da[i], "xb": db[i], "ident": eye} for i in range(NCORES)],
        core_ids=list(range(NCORES)),
        trace=trace,
    )
    out = np.empty((NCORES, P, JPP, 7), np.float32)
    for i, r in enumerate(res.results):
        out[i, :, :, 0:3] = r["ya"].reshape(P, 3, JPP).transpose(0, 2, 1)
        out[i, :, :, 3:7] = r["yb"].reshape(P, 4, JPP).transpose(0, 2, 1)
    return out.reshape(NCORES * NC_ELEMS, 7)[:B], res


def kernel(affine):
    y, _ = _run(np.asarray(affine), trace=False)
    return y


# revision 39
# speedup vs baseline: 1.5028x; 1.0637x over previous
"""nn_AffineLog: batched 4x4 affine matrix-log projected onto the 7-dim CSO basis.

Closed-form algorithm (replaces the reference's 24-term Mercator series):
inputs are exactly [[e^s R, t],[0,1]] with R a rotation, so
  L3x3 = s I + f (R - R^T),  f = asin(x)/(2x), x = sin th  (poly in x^2)
  translation u' = psi(C) t, psi(x) = x/(e^x-1), reduced via
  Omega^3 = -th^2 Omega to u' = (A - D q) t + B (w x t) + D (w.t) w.

Data-parallel over 8 NeuronCores. The host packs the 10 live channels of
each affine into channel-planar per-partition DRAM, so every DVE access is
contiguous; elementwise pipeline on DVE/ACT with custom fused DVE ops.
"""

import os

os.environ.setdefault("BY_DEFAULT_DISABLE_SUBTILE_DEPS", "1")

import functools
from contextlib import ExitStack

import numpy as np

import concourse.bass as bass
import concourse.bacc as bacc
import concourse.hw_specs as hw_specs
import concourse.mybir as mybir
from concourse.tile import TileContext
from concourse.bass_utils import run_bass_kernel_spmd
from concourse import dve_ops as dops
from concourse.dve_spec import (
    Spec, Src0, Src1, C0, C1, C2, C3, One, sq, _spill_c3_to_src1, lower,
    _has_src1,
)
from concourse.dve_uop import DveOpSpec

AF = mybir.ActivationFunctionType
OP = mybir.AluOpType
F32 = mybir.dt.float32

NCORES = 8
B = 2_000_000
P = 128
JPP = 1954                  # free-dim elements per partition per core
NC_ELEMS = P * JPP          # 250112 per core (total 2000896, pad 896)
TILES = (512, 512, 512, 418)

# packed channel order (host): [m01, m10, m02, m20, m12, m21, m00] + [t0, t1, t2]
CH_A = [1, 4, 2, 8, 6, 9, 0]   # 7 "matrix" planes -> tensor xa
CH_B = [3, 7, 11]              # 3 translation planes -> tensor xb

SQ2 = float(np.sqrt(2.0))
SQ3 = float(np.sqrt(3.0))
# f'(z) = 2*asin(x)/(2x) with z = 4x^2:  f' = 1 + c1 z + c2 z^2 + c3 z^3 + c4 z^4
FP_C1 = 1.0 / 24.0
FP_C2 = 2.0 * 0.5 * (3.0 / 40.0) / 16.0
FP_C3 = 2.0 * 0.5 * (5.0 / 112.0) / 64.0
FP_C4 = 2.0 * 0.5 * (35.0 / 1152.0) / 256.0

# Restrict ACT table choice to the one set holding ln+exp+copy, so bacc
# never alternates table loads between tiles. Other set names stay (ids are
# positional) but advertise no functions.
_orig_gat = hw_specs.get_activation_tables


@functools.cache
def _gat_ln_exp_only(module_arch):
    t = _orig_gat(module_arch)
    keep = "natural_log_exp_and_others"
    return {k: (v if k == keep else set()) for k, v in t.items()}


hw_specs.get_activation_tables = _gat_ln_exp_only
bacc.get_activation_tables = _gat_ln_exp_only


# --- custom fused DVE ops (registered into concourse.dve_ops at import) ----
def _register(name, body):
    if name in dops._SUB_OPCODE_FOR_NAME:
        return next(o for o in dops.OPS if o.name == name)
    dops._SUB_OPCODE_FOR_NAME[name] = dops._CUSTOM_DVE_ROW_BASE + len(dops.OPS)
    assert dops._SUB_OPCODE_FOR_NAME[name] < 0x20
    spec = Spec(body=body)
    lowered = DveOpSpec(
        name=name,
        opcode=dops._SUB_OPCODE_FOR_NAME[name],
        uops=lower(spec, ver="v3"),
        rd1_en=_has_src1(spec),
    )
    op = dops.DveOp(name=name, spec=spec, subdim=False,
                    uops_sha={"v3": lowered.sha("v3")})
    dops.OPS.append(op)
    dops.CUSTOM_DVE_SPECS[name] = spec
    return op


OP_SQSUM = _register("ANT_AFL_SQSUM", sq(Src0) + sq(Src1))
OP_ADDSQ = _register("ANT_AFL_ADDSQ", Src0 + sq(Src1))
OP_POLY4 = _register(
    "ANT_AFL_POLY4",
    _spill_c3_to_src1(((((Src0 * C0 + C1) * Src0 + C2) * Src0 + C3) * Src0) + One),
)
_m2 = (Src0 * C0) * Src0
# Ap = A(s) - Src1  (Src1 = D*qt folded in, saving a separate subtract)
OP_APCOEF = _register(
    "ANT_AFL_APCOEF", ((((_m2 + C1) * Src0 + C2) * Src0) + One) - Src1)
_s2 = Src0 * Src0
OP_BCOEF = _register(
    "ANT_AFL_BCOEF", (Src0 * C1 + C2) + ((_s2 * Src0 - Src0 * Src1) * C0))
OP_DCOEF = _register(
    "ANT_AFL_DCOEF", ((Src0 * Src0) * C0 + Src1 * C1) + C2)
OP_QTH = _register("ANT_AFL_QTH", (sq(Src0) * Src1) * C0)
OP_DG2 = _register("ANT_AFL_DG2", Src0 * sq(Src1))


def _build(jpp=JPP, tiles=TILES):
    nc = bacc.Bacc("TRN2", target_bir_lowering=False, debug=False)
    xa = nc.dram_tensor("xa", (P, 7 * jpp), F32, kind="ExternalInput")
    xb = nc.dram_tensor("xb", (P, 3 * jpp), F32, kind="ExternalInput")
    ident = nc.dram_tensor("ident", (P, P), F32, kind="ExternalInput")
    ya = nc.dram_tensor("ya", (P, 3 * jpp), F32, kind="ExternalOutput")
    yb = nc.dram_tensor("yb", (P, 4 * jpp), F32, kind="ExternalOutput")
    xav = xa[:, :].rearrange("p (c j) -> p c j", j=jpp)
    xbv = xb[:, :].rearrange("p (c j) -> p c j", j=jpp)
    yav = ya[:, :].rearrange("p (c j) -> p c j", j=jpp)
    ybv = yb[:, :].rearrange("p (c j) -> p c j", j=jpp)

    mul, add, sub = OP.mult, OP.add, OP.subtract

    with TileContext(nc) as tc:
        with (
            tc.tile_pool(name="cst", bufs=1) as cstp,
            tc.tile_pool(name="io", bufs=2) as iop,
            tc.tile_pool(name="tp", bufs=1) as tp,
            tc.tile_pool(name="ps", bufs=1, space="PSUM") as psp,
        ):
            c1col = cstp.tile([P, 1], F32, name="c1col")
            nc.vector.memset(c1col, FP_C1)
            IDT = cstp.tile([P, P], F32, name="IDT")
            nc.sync.dma_start(out=IDT, in_=ident[:, :])
            IDTN = cstp.tile([P, P], F32, name="IDTN")
            nc.scalar.mul(IDTN, IDT, -1.0)

            off = 0
            for tix, nf in enumerate(tiles):
                pe_tile = tix != len(tiles) - 1
                INA = iop.tile([P, nf * 7], F32, tag="ina", name="tina")
                INB = iop.tile([P, nf * 3], F32, tag="inb", name="tinb")
                OUTA = iop.tile([P, nf * 3], F32, tag="outa", name="touta")
                OUTB = iop.tile([P, nf * 4], F32, tag="outb", name="toutb")
                nc.sync.dma_start(
                    out=INA.rearrange("p (c j) -> p c j", c=7),
                    in_=xav[:, :, off:off + nf])
                nc.sync.dma_start(
                    out=INB.rearrange("p (c j) -> p c j", c=3),
                    in_=xbv[:, :, off:off + nf])

                def T(nm, k=1):
                    return tp.tile([P, nf * k], F32, tag=nm, name=nm)

                def pl(t, i, k=1):
                    return t[:, i * nf:(i + k) * nf]

                def pl3(t, i=0):
                    return t[:, i * nf:(i + 3) * nf].rearrange(
                        "p (c j) -> p c j", c=3)

                def bc3(a):
                    return a.rearrange("p (o j) -> p o j", o=1).to_broadcast(
                        [P, 3, nf])

                def tt(o, a, b, op):
                    nc.vector.tensor_tensor(out=o, in0=a, in1=b, op=op)

                def stt(o, a, s, b, op0, op1):
                    nc.vector.scalar_tensor_tensor(
                        out=o, in0=a, scalar=s, in1=b, op0=op0, op1=op1)

                def cust(op_, o, a, b=None, s0=0.0, s1=0.0, imm2=0.0):
                    nc.vector._custom_dve(
                        op_, out=o, in0=a, in1=b, s0=s0, s1=s1, imm2=imm2)

                tv = pl3(INB)  # [p, 3, nf] translation planes

                u = T("u"); v = T("v")
                # e^{2s} = m00^2 + m10^2 + m20^2  (planes 6, 1, 3 of INA)
                cust(OP_SQSUM, u, pl(INA, 6), pl(INA, 1))
                e2s = T("e2s")
                cust(OP_ADDSQ, e2s, u, pl(INA, 3))
                lnd2 = T("lnd2"); es = T("es"); es2 = T("es2"); s = T("s")
                nc.scalar.activation(out=lnd2, in_=e2s, func=AF.Ln)
                nc.scalar.activation(out=es, in_=lnd2, func=AF.Exp, scale=-0.5)
                nc.scalar.activation(out=es2, in_=lnd2, func=AF.Exp, scale=-1.0)
                nc.scalar.mul(s, lnd2, 0.5)
                nc.scalar.mul(pl(OUTB, 3), lnd2, SQ3 / 2.0)   # out6

                # A3/P9 feed the PE accumulation at tile end; double-buffer
                # them so the next tile's DVE work never waits on PE drain.
                A3 = tp.tile([P, nf * 3], F32, tag="A3", name="A3", bufs=2)
                tt(pl(A3, 0), pl(INA, 0), pl(INA, 1), sub)   # a1 = m01 - m10
                tt(pl(A3, 1), pl(INA, 2), pl(INA, 3), sub)   # a2 = m02 - m20
                tt(pl(A3, 2), pl(INA, 4), pl(INA, 5), sub)   # a3 = m12 - m21
                cust(OP_SQSUM, v, pl(A3, 0), pl(A3, 1))
                S = T("S")
                cust(OP_ADDSQ, S, v, pl(A3, 2))
                # all 9 products P[i,j] = a_i * t_j at plane 3i+j
                P9 = tp.tile([P, nf * 9], F32, tag="P9", name="P9", bufs=2)
                for i in range(3):
                    tt(pl3(P9, 3 * i), bc3(pl(A3, i)), tv, mul)
                # ctil/dtil combines: on PE tiles these run as +/- identity
                # matmul accumulations into PSUM (idle engine); on the last
                # tile they stay on the DVE to keep the kernel tail short.
                if pe_tile:
                    def _mmsum(pstag, terms):
                        ps = psp.tile([P, nf], F32, tag=pstag, name=pstag)
                        for k, (sgn, src) in enumerate(terms):
                            nc.tensor.matmul(
                                ps[:, :], (IDT if sgn > 0 else IDTN)[:, :],
                                src, start=(k == 0),
                                stop=(k == len(terms) - 1))
                        return ps
                    csx = _mmsum("csx", [(1, pl(P9, 1)), (1, pl(P9, 5))])
                    csy = _mmsum("csy", [(1, pl(P9, 8)), (-1, pl(P9, 0))])
                    csz = _mmsum("csz", [(-1, pl(P9, 7)), (-1, pl(P9, 3))])
                    dts = _mmsum("dts", [(1, pl(P9, 4)), (-1, pl(P9, 6)),
                                         (-1, pl(P9, 2))])
                    dt = dts[:, :]
                    cpl = (csx[:, :], csy[:, :], csz[:, :])
                else:
                    dA = T("dA"); dtt = T("dt")
                    tt(dA, pl(P9, 4), pl(P9, 6), sub)           # a2t1-a3t0
                    tt(dtt, dA, pl(P9, 2), sub)                 # - a1t2
                    dt = dtt
                    tt(pl(P9, 2), pl(P9, 1), pl(P9, 5), add)        # cx
                    tt(pl(P9, 4), pl(P9, 8), pl(P9, 0), sub)        # cy
                    stt(pl(P9, 6), pl(P9, 7), -1.0, pl(P9, 3), mul, sub)
                # scalar chain (ACT outputs ready by now)
                z = T("z")
                tt(z, es2, S, mul)                  # z = 4 sin^2 th
                fp = T("fp")
                cust(OP_POLY4, fp, z, c1col, s0=FP_C4, s1=FP_C3, imm2=FP_C2)
                qt = T("qt")
                cust(OP_QTH, qt, fp, z, s0=0.25)    # th^2
                g = T("g")
                stt(g, fp, 0.5, es, mul, mul)       # g = f e^{-s}
                # rotation outputs = sqrt2 * g * a_k -> OUTB planes 0..2
                stt(pl3(OUTB), bc3(g), SQ2, pl3(A3), mul, mul)
                nc.sync.dma_start(
                    out=ybv[:, :, off:off + nf],
                    in_=OUTB.rearrange("p (c j) -> p c j", c=4))
                # psi coefficients (slots reuse dead temps)
                Bc = T("S"); D = T("lnd2")
                cust(OP_BCOEF, Bc, s, qt,
                     s0=-1.0 / 180.0, s1=1.0 / 6.0, imm2=-0.5)
                cust(OP_DCOEF, D, s, qt,
                     s0=-1.0 / 120.0, s1=1.0 / 720.0, imm2=1.0 / 12.0)
                v2 = T("u"); Ap = T("es2"); Bg = T("s"); Dg2 = T("fp")
                tt(v2, D, qt, mul)
                cust(OP_APCOEF, Ap, s, v2,
                     s0=-1.0 / 720.0, s1=1.0 / 12.0, imm2=-0.5)
                # w1 product as early as possible so PE can start its
                # accumulation while the DVE computes w2/pw.
                W1 = T("W1", 3)
                tt(pl3(W1), bc3(Ap), tv, mul)
                tt(Bg, Bc, g, mul)
                cust(OP_DG2, Dg2, D, g)
                P3 = T("z")
                tt(P3, Dg2, dt, mul)
                # pw = P3*(-a3,+a2,-a1) into free P9 planes 1,3,5
                stt(pl(P9, 1), P3, -1.0, pl(A3, 2), mul, mul)
                tt(pl(P9, 3), P3, pl(A3, 1), mul)
                stt(pl(P9, 5), P3, -1.0, pl(A3, 0), mul, mul)
                # w2 = Bg*ctil (into A3, fully consumed)
                pwview = P9[:, 1 * nf:7 * nf].rearrange(
                    "p (c t j) -> p c t j", c=3, t=2)[:, :, 0, :]
                if pe_tile:
                    for i in range(3):
                        tt(pl(A3, i), Bg, cpl[i], mul)
                else:
                    cview = P9[:, 2 * nf:8 * nf].rearrange(
                        "p (c t j) -> p c t j", c=3, t=2)[:, :, 0, :]
                    tt(pl3(A3), bc3(Bg), cview, mul)
                # sum w1 + w2 + pw. PE tiles: identity-weight matmuls
                # accumulate the three terms per plane in PSUM (idle PE), ACT
                # copies PSUM -> SBUF — frees 6 DVE units and overlaps the
                # next tile. Last tile: plain DVE adds — the serial
                # PE chain would lengthen the kernel tail.
                if pe_tile:
                    for i in range(3):
                        PSi = psp.tile([P, nf], F32, tag=f"ps{i}", name=f"ps{i}")
                        pwsrc = pl(P9, 1 + 2 * i)
                        terms = (pl(W1, i), pl(A3, i), pwsrc)
                        for c0 in range(0, nf, 512):
                            w = min(512, nf - c0)
                            for k, src in enumerate(terms):
                                nc.tensor.matmul(
                                    PSi[:, c0:c0 + w],
                                    IDT[:, :], src[:, c0:c0 + w],
                                    start=(k == 0), stop=(k == 2))
                        nc.scalar.copy(pl(OUTA, i), PSi[:, :])
                        nc.sync.dma_start(
                            out=yav[:, i, off:off + nf], in_=pl(OUTA, i))
                else:
                    tt(pl3(W1), pl3(W1), pl3(A3), add)
                    tt(pl3(OUTA), pl3(W1), pwview, add)
                    nc.sync.dma_start(
                        out=yav[:, :, off:off + nf],
                        in_=OUTA.rearrange("p (c j) -> p c j", c=3))
                off += nf
    if not nc.is_finalized():
        nc.finalize()
    return nc


def _pack(affine):
    """(B,4,4) f32 -> per-core channel-planar arrays xa (P,7*jpp), xb (P,3*jpp)."""
    x = np.ascontiguousarray(affine.reshape(B, 16).astype(np.float32, copy=False))
    pad = NCORES * NC_ELEMS - B
    padblk = np.zeros((pad, 16), np.float32)
    padblk[:, [0, 5, 10, 15]] = 1.0  # identity affines -> log = 0
    data = np.concatenate([x, padblk], 0).reshape(NCORES, P, JPP, 16)
    da = np.ascontiguousarray(data[:, :, :, CH_A].transpose(0, 1, 3, 2))
    db = np.ascontiguousarray(data[:, :, :, CH_B].transpose(0, 1, 3, 2))
    return (da.reshape(NCORES, P, 7 * JPP), db.reshape(NCORES, P, 3 * JPP))


def _run(affine, trace=False):
    da, db = _pack(np.asarray(affine))
    nc = _build()
    eye = np.ascontiguousarray(np.eye(P, dtype=np.float32))
    res = run_bass_kernel_spmd(
        nc,
        [{"xa": da[i], "xb": db[i], "ident": eye} for i in range(NCORES)],
        core_ids=list(range(NCORES)),
        trace=trace,
    )
    out = np.empty((NCORES, P, JPP, 7), np.float32)
    for i, r in enumerate(res.results):
        out[i, :, :, 0:3] = r["ya"].reshape(P, 3, JPP).transpose(0, 2, 1)
        out[i, :, :, 3:7] = r["yb"].reshape(P, 4, JPP).transpose(0, 2, 1)
    return out.reshape(NCORES * NC_ELEMS, 7)[:B], res


def kernel(affine):
    y, _ = _run(np.asarray(affine), trace=False)
    return y
